# revision 56
# baseline (speedup 1.0000x reference)
"""Trainium2 Bass kernel for nn_DeltaLag (LSTM encoder + lagged cross-attention
top-k + MLP head), distributed over 8 NeuronCores.

Sharding: stocks are split 375/core (LSTM + keys local to each core); every
core computes the score block [3072 padded target positions x 3750 local
(stock,lag) columns] in fp32, takes its local top-8 per target, and candidate
(value, index) pairs are exchanged with three pipelined AllToAlls (one per
8-tile position group) so the exchange and the per-group merge overlap the
next group's score computation. Each core merges + finishes its own 375
targets (z-gather + softmax + MLP).

Position layout: target t (owner d = t//375, i = t%375, g = i//125, o=i%125)
lives at position g*1024 + d*128 + o, i.e. tile T = 8g + d, row o. A group's
AllGather over rows [g*1024, (g+1)*1024) delivers core d's targets'
candidates from every source core at rows s*1024 + d*128 + o.

The compiled program is identical on all 8 cores (SPMD); everything
device-specific (shards, self-column ids, gather indices) is passed as input
tensors. All matmuls run in true fp32 (fp32r measured at ~1e-3 relative error
on this hardware, which would flip top-k selections).
"""

import sys

sys.path.insert(0, "/opt/trn_rl_repo")

import numpy as np

import concourse.bacc as bacc
import concourse.mybir as mybir
import concourse.tile as tile
from concourse.bass import IndirectOffsetOnAxis
from concourse.bass_utils import run_bass_kernel_spmd
from concourse.masks import make_identity

F32 = mybir.dt.float32
U32 = mybir.dt.uint32
U16 = mybir.dt.uint16
AF = mybir.ActivationFunctionType
ALU = mybir.AluOpType

S, F, N, L, LMAX, K = 3000, 6, 128, 40, 10, 5
ND = 8                      # cores
SS = S // ND                # stocks per core
COLS = SS * LMAX            # score columns per core
NG = 3                      # candidate-exchange groups
GPOS = ND * 128             # positions per group (1024)
NPOS = NG * GPOS            # padded target count (3072)
NT = NPOS // 128            # target tiles (24)
GT = NT // NG               # tiles per group (8)
MTS = 125                   # used rows per (group, owner) slot
SCH = 1024                  # score-tile PSUM chunk width
XCH = 8                     # xt DMA chunks (5 timesteps each)
CW16 = 16                   # u32 words per exchanged candidate row (8 v + 8 idx)


def build_program(aw):
    """aw: per-core stock-index threshold splitting the q AllGather into an
    early small exchange (rows [0,aw), enough for group-0's targets) and a
    late one hidden under group-0's score scans."""
    assert 125 <= aw <= 250
    bw = SS - aw
    nc = bacc.Bacc("TRN2", target_bir_lowering=False, debug=False,
                   enable_asserts=True, num_devices=ND)

    # ---- I/O ----
    d_xt = nc.dram_tensor("xt", [F + 1, L * SS], F32, kind="ExternalInput")
    d_wih = nc.dram_tensor("wih_t", [F + 1, 4 * N], F32, kind="ExternalInput")
    d_whh = nc.dram_tensor("whh_t", [N, 4 * N], F32, kind="ExternalInput")
    d_wqt = nc.dram_tensor("wq_t", [N, N], F32, kind="ExternalInput")
    d_wkt = nc.dram_tensor("wk_t", [N, N], F32, kind="ExternalInput")
    d_nuq = nc.dram_tensor("negu_q", [1, N], F32, kind="ExternalInput")
    d_nuk = nc.dram_tensor("negu_k", [1, N], F32, kind="ExternalInput")
    d_invt = nc.dram_tensor("invt", [1, 1], F32, kind="ExternalInput")
    d_gslo = nc.dram_tensor("gslo", [128, NG], F32, kind="ExternalInput")
    d_trowsA = nc.dram_tensor("tgtrowsA", [128, NT], U32, kind="ExternalInput")
    d_trowsB = nc.dram_tensor("tgtrowsB", [128, NT], U32, kind="ExternalInput")
    d_xzb = nc.dram_tensor("xzb", [S * LMAX, 8], F32, kind="ExternalInput")
    d_mrows = nc.dram_tensor("mrows", [128, NG * ND], U32, kind="ExternalInput")
    d_orows = nc.dram_tensor("orows", [128, NG], U32, kind="ExternalInput")
    d_w1t = nc.dram_tensor("w1_t", [2 * F, 64], F32, kind="ExternalInput")
    d_w2t = nc.dram_tensor("w2_t", [64, 32], F32, kind="ExternalInput")
    d_w3t = nc.dram_tensor("w3_t", [32, 1], F32, kind="ExternalInput")
    d_b1 = nc.dram_tensor("b1c", [64, 1], F32, kind="ExternalInput")
    d_b2 = nc.dram_tensor("b2c", [32, 1], F32, kind="ExternalInput")
    d_b3 = nc.dram_tensor("b3c", [1, 1], F32, kind="ExternalInput")

    d_y = nc.dram_tensor("y", [S, 1], F32, kind="ExternalOutput")

    d_qblA = nc.dram_tensor("qb_localA", [aw, N], F32)
    d_qblB = nc.dram_tensor("qb_localB", [bw, N], F32)
    d_qbaA = nc.dram_tensor("qb_allA", [ND * aw, N], F32, addr_space="Shared")
    d_qbaB = nc.dram_tensor("qb_allB", [ND * bw, N], F32, addr_space="Shared")
    d_clg = [nc.dram_tensor(f"cand_local{g}", [GPOS, CW16], U32)
             for g in range(NG)]
    d_cag = [nc.dram_tensor(f"cand_all{g}", [ND * GPOS, CW16], U32,
                            addr_space="Shared") for g in range(NG)]

    groups = [list(range(ND))]

    with tile.TileContext(nc) as tc:
        cpool = tc.alloc_tile_pool(name="const", bufs=1)
        big = tc.alloc_tile_pool(name="big", bufs=1)

        # ---- constants / params to SBUF ----
        ident = cpool.tile([128, 128], F32)
        make_identity(nc, ident[:])
        ones1 = cpool.tile([1, 128], F32)
        nc.vector.memset(ones1[:], 1.0)
        onesf = cpool.tile([128, 128], F32)
        nc.vector.memset(onesf[:], 1.0)

        def load(pool, dram, shape, dtype=F32):
            t = pool.tile(shape, dtype, tag=f"ld_{dram.name}")
            nc.sync.dma_start(out=t[:], in_=dram[:, :])
            return t

        # LSTM weights and the ppre inputs first (the SP DMA queue is
        # in-order; PE's first scheduled ops are the ppre matmuls and the
        # first LSTM step, which need these plus only the first xt chunk)
        wih = load(cpool, d_wih, [F + 1, 4 * N])
        whh = load(cpool, d_whh, [N, 4 * N])
        nuq = load(cpool, d_nuq, [1, N])
        nuk = load(cpool, d_nuk, [1, N])
        # time-major xt arrives in XCH separate chunk tiles so the LSTM's
        # step-t matmul depends only on its own chunk's DMA
        TPC = L // XCH
        CW = TPC * SS
        xts = []
        for c in range(XCH):
            xtc = big.tile([F + 1, CW], F32, tag=f"xt{c}")
            nc.sync.dma_start(out=xtc[:], in_=d_xt[:, c * CW:(c + 1) * CW])
            xts.append(xtc)
        wqt = load(cpool, d_wqt, [N, N])
        wkt = load(cpool, d_wkt, [N, N])
        invt = load(cpool, d_invt, [1, 1])
        trowsA = load(cpool, d_trowsA, [128, NT], U32)
        trowsB = load(cpool, d_trowsB, [128, NT], U32)
        mrows = load(cpool, d_mrows, [128, NG * ND], U32)
        orows = load(cpool, d_orows, [128, NG], U32)
        w1t = load(cpool, d_w1t, [2 * F, 64])
        w2t = load(cpool, d_w2t, [64, 32])
        w3t = load(cpool, d_w3t, [32, 1])
        b1c = load(cpool, d_b1, [64, 1])
        b2c = load(cpool, d_b2, [32, 1])
        b3c = load(cpool, d_b3, [1, 1])

        gslo = load(cpool, d_gslo, [128, NG])
        invtb = cpool.tile([128, 1], F32)
        nc.gpsimd.partition_broadcast(invtb[:], invt[:], channels=128)

        iota_u = cpool.tile([128, 64], U32)
        nc.gpsimd.iota(iota_u[:], pattern=[[1, 64]], base=0, channel_multiplier=0)
        iota_f = cpool.tile([128, 64], F32)
        nc.vector.tensor_copy(iota_f[:], iota_u[:])
        base_u = cpool.tile([128, 64], U32)
        nc.gpsimd.iota(base_u[:], pattern=[[COLS, 8], [0, 8]], base=0,
                       channel_multiplier=0)
        base_f = cpool.tile([128, 64], F32)
        nc.vector.tensor_copy(base_f[:], base_u[:])

        # rank-1 LN-fold correction matrices: rows n, cols p -> -u[p]/128
        with tc.tile_pool(name="ppre", bufs=1, space="PSUM") as ppre:
            uqo = cpool.tile([128, 128], F32)
            uko = cpool.tile([128, 128], F32)
            pq = ppre.tile([128, 128], F32, space="PSUM")
            nc.tensor.matmul(out=pq[:], lhsT=ones1[:], rhs=nuq[:], start=True, stop=True)
            nc.scalar.activation(uqo[:], pq[:], AF.Copy)
            pk = ppre.tile([128, 128], F32, space="PSUM")
            nc.tensor.matmul(out=pk[:], lhsT=ones1[:], rhs=nuk[:], start=True, stop=True)
            nc.scalar.activation(uko[:], pk[:], AF.Copy)

        # ---- Phase 1: LSTM over the 375 local stocks ----
        # h,c layout [n=128, s]; last-10 hidden states land in hsave[n, s*10+k].
        # Gate columns in wih/whh are host-permuted to [i, f, o, g]; the bias
        # is folded into the xproj matmul via xt's constant-1 row.
        hsave = big.tile([128, COLS], F32)
        with tc.tile_pool(name="lstm_sb", bufs=2) as lsb, \
             tc.tile_pool(name="lstm_c", bufs=2) as lcp, \
             tc.tile_pool(name="lstm_ps", bufs=2, space="PSUM") as lps:
            h0 = lsb.tile([128, SS], F32, tag="h0")
            nc.vector.memset(h0[:], 0.0)
            c_prev = lcp.tile([128, SS], F32, tag="c")
            nc.vector.memset(c_prev[:], 0.0)
            h_prev = h0[:]
            for t in range(L):
                xs = xts[t // TPC][:, (t % TPC) * SS:(t % TPC + 1) * SS]
                # all four x-projections first: they don't depend on h_{t-1},
                # so the PE has work while the gate elementwise chain finishes
                gps = []
                for g in range(4):
                    psg = lps.tile([128, SS], F32, tag=f"g{g}", space="PSUM")
                    nc.tensor.matmul(out=psg[:], lhsT=wih[:, g * N:(g + 1) * N],
                                     rhs=xs, start=True, stop=False)
                    gps.append(psg)
                for g in range(4):
                    nc.tensor.matmul(out=gps[g][:], lhsT=whh[:, g * N:(g + 1) * N],
                                     rhs=h_prev, start=False, stop=True)
                si = lsb.tile([128, SS], F32, tag="si")
                nc.scalar.activation(si[:], gps[0][:], AF.Sigmoid)
                sf = lsb.tile([128, SS], F32, tag="sf")
                nc.scalar.activation(sf[:], gps[1][:], AF.Sigmoid)
                so_t = lsb.tile([128, SS], F32, tag="so")
                nc.scalar.activation(so_t[:], gps[2][:], AF.Sigmoid)
                tg = lsb.tile([128, SS], F32, tag="tg")
                nc.scalar.activation(tg[:], gps[3][:], AF.Tanh)
                si, sf, so = si[:], sf[:], so_t[:]
                t1 = lsb.tile([128, SS], F32, tag="t1")
                nc.vector.tensor_tensor(out=t1[:], in0=si, in1=tg[:], op=ALU.mult)
                c2 = lsb.tile([128, SS], F32, tag="c2")
                nc.gpsimd.tensor_tensor(out=c2[:], in0=sf, in1=c_prev[:], op=ALU.mult)
                c_new = lcp.tile([128, SS], F32, tag="c")
                nc.vector.tensor_tensor(out=c_new[:], in0=c2[:], in1=t1[:], op=ALU.add)
                th = lsb.tile([128, SS], F32, tag="th")
                nc.scalar.activation(th[:], c_new[:], AF.Tanh)
                if t >= L - LMAX:
                    h_out = hsave[:, (t - (L - LMAX))::LMAX]
                    nc.gpsimd.tensor_tensor(out=h_out, in0=so, in1=th[:], op=ALU.mult)
                    h_prev = h_out
                else:
                    hn = lsb.tile([128, SS], F32, tag="hn")
                    nc.gpsimd.tensor_tensor(out=hn[:], in0=so, in1=th[:], op=ALU.mult)
                    h_prev = hn[:]
                c_prev = c_new

        # ---- Phase 2: queries first, so the AllGather overlaps the keys ----
        with tc.tile_pool(name="q_sb", bufs=2) as qsb, \
             tc.tile_pool(name="q_ps", bufs=2, space="PSUM") as qps:
            h39 = hsave[:, (LMAX - 1)::LMAX]
            pyq = qps.tile([128, SS], F32, tag="yq", space="PSUM")
            nc.tensor.matmul(out=pyq[:], lhsT=wqt[:], rhs=h39, start=True, stop=False)
            nc.tensor.matmul(out=pyq[:], lhsT=uqo[:], rhs=h39, start=False, stop=True)
            yq = qsb.tile([128, SS], F32, tag="yq_sb")
            nc.scalar.activation(yq[:], pyq[:], AF.Copy)
            y2q = qsb.tile([128, SS], F32, tag="y2q")
            nc.scalar.activation(y2q[:], pyq[:], AF.Square)
            psq = qps.tile([128, SS], F32, tag="sq", space="PSUM")
            nc.tensor.matmul(out=psq[:], lhsT=onesf[:], rhs=y2q[:], start=True, stop=True)
            sq = qsb.tile([128, SS], F32, tag="sqq")
            nc.scalar.sqrt(sq[:], psq[:])
            ri = qsb.tile([128, SS], F32, tag="riq")
            nc.vector.reciprocal(ri[:], sq[:])
            qn1 = qsb.tile([128, SS], F32, tag="qn1")
            nc.vector.tensor_tensor(out=qn1[:], in0=yq[:], in1=ri[:], op=ALU.mult)
            qn = qsb.tile([128, SS], F32, tag="qn")
            nc.vector.tensor_scalar(out=qn[:], in0=qn1[:], scalar1=invtb[:, 0:1],
                                    scalar2=None, op0=ALU.mult)
            for j in range(3):
                pt = qps.tile([128, 128], F32, tag="qt", space="PSUM")
                nc.tensor.transpose(out=pt[0:MTS, :], in_=qn[:, j * MTS:(j + 1) * MTS],
                                    identity=ident[:])
                qrow_sb = qsb.tile([128, 128], F32, tag="qrow_sb")
                nc.scalar.activation(qrow_sb[0:MTS, :], pt[0:MTS, :], AF.Copy)
                # split the q rows at local stock index `aw` between the
                # early (A) and late (B) exchange tensors
                lo, hi = j * MTS, j * MTS + MTS
                if hi <= aw:
                    nc.sync.dma_start(out=d_qblA[lo:hi, :], in_=qrow_sb[0:MTS, :])
                elif lo >= aw:
                    nc.sync.dma_start(out=d_qblB[lo - aw:hi - aw, :],
                                      in_=qrow_sb[0:MTS, :])
                else:
                    na = aw - lo
                    nc.sync.dma_start(out=d_qblA[lo:aw, :], in_=qrow_sb[0:na, :])
                    nc.sync.dma_start(out=d_qblB[0:hi - aw, :],
                                      in_=qrow_sb[na:MTS, :])
                if hi >= aw and lo < aw:
                    # all A rows written: fire the early exchange now
                    nc.gpsimd.collective_compute(
                        "AllGather", ALU.bypass, replica_groups=groups,
                        ins=[d_qblA.ap().opt()], outs=[d_qbaA.ap().opt()])

        # ---- Phase 3: keys (LN+l2norm folded into matmuls; sigma cancels) ----
        keysT = big.tile([128, COLS], F32)
        with tc.tile_pool(name="key_sb", bufs=3) as ksb, \
             tc.tile_pool(name="key_ps", bufs=2, space="PSUM") as kps:
            ysb = big.tile([128, COLS], F32)
            CH = 512
            chunks = [(c0, min(COLS, c0 + CH)) for c0 in range(0, COLS, CH)]
            sqs = []
            for c0, c1 in chunks:
                w = c1 - c0
                py = kps.tile([128, CH], F32, tag="y", space="PSUM")
                nc.tensor.matmul(out=py[:, :w], lhsT=wkt[:],
                                 rhs=hsave[:, c0:c1], start=True, stop=False)
                nc.tensor.matmul(out=py[:, :w], lhsT=uko[:],
                                 rhs=hsave[:, c0:c1], start=False, stop=True)
                nc.scalar.activation(ysb[:, c0:c1], py[:, :w], AF.Copy)
                y2 = ksb.tile([128, CH], F32, tag="y2")
                nc.scalar.activation(y2[:, :w], py[:, :w], AF.Square)
                psq = kps.tile([128, CH], F32, tag="s", space="PSUM")
                nc.tensor.matmul(out=psq[:, :w], lhsT=onesf[:],
                                 rhs=y2[:, :w], start=True, stop=True)
                sq = ksb.tile([128, CH], F32, tag="sq")
                nc.scalar.sqrt(sq[:, :w], psq[:, :w])
                sqs.append((sq, c0, c1))
            for sq, c0, c1 in sqs:
                w = c1 - c0
                ri = ksb.tile([128, CH], F32, tag="ri")
                nc.vector.reciprocal(ri[:, :w], sq[:, :w])
                nc.vector.tensor_tensor(out=keysT[:, c0:c1], in0=ysb[:, c0:c1],
                                        in1=ri[:, :w], op=ALU.mult)

        # ---- Phase 4+5+6: per group: qT gather/transpose + scores + top-8
        # (per tile), candidate AllGather exchange, and the merge of the
        # PREVIOUS group's exchanged candidates ----
        qT_all = big.tile([128, NPOS], F32)
        v8 = big.tile([128, NT * 8], F32)
        i8 = big.tile([128, NT * 8], U32)
        cl_v = [d_clg[g].ap().rearrange("(T p) w -> p T w", p=128)
                for g in range(NG)]

        gsb = tc.alloc_tile_pool(name="qg_sb", bufs=3)
        gps_ = tc.alloc_tile_pool(name="qg_ps", bufs=2, space="PSUM")
        ssb_p = tc.alloc_tile_pool(name="sc_sb", bufs=2)
        sps = tc.alloc_tile_pool(name="sc_ps", bufs=2, space="PSUM")
        msb = tc.alloc_tile_pool(name="m_sb", bufs=2)
        mps = tc.alloc_tile_pool(name="m_ps", bufs=1, space="PSUM")

        def merge_group(g):
            # candidates for my 125 targets of group g, from all 8 cores
            mv = msb.tile([128, ND * CW16], U32, tag="mv")
            for s in range(ND):
                nc.gpsimd.indirect_dma_start(
                    out=mv[0:MTS, s * CW16:(s + 1) * CW16], out_offset=None,
                    in_=d_cag[g][:, :],
                    in_offset=IndirectOffsetOnAxis(
                        ap=mrows[0:MTS, g * ND + s:g * ND + s + 1], axis=0))
            mvals = mv[0:MTS, :].bitcast(F32).rearrange(
                "p (d w) -> p d w", w=CW16)[:, :, 0:8]
            midx = mv[0:MTS, :].rearrange("p (d w) -> p d w", w=CW16)[:, :, 8:16]
            # leading copies + self-filter arithmetic on gpsimd so the DVE's
            # scan stream is disturbed as little as possible
            mvalc = msb.tile([128, 64], F32, tag="mvalc")
            nc.gpsimd.tensor_copy(mvalc[0:MTS, :], mvals)
            # global flat candidate index = owner*COLS + local
            midxf = msb.tile([128, 64], F32, tag="midxf")
            nc.gpsimd.tensor_copy(midxf[0:MTS, :], midx)
            nc.gpsimd.tensor_tensor(out=midxf[0:MTS, :], in0=midxf[0:MTS, :],
                                    in1=base_f[0:MTS, :], op=ALU.add)
            # self-stock filter (candidates were exchanged unmasked):
            # self iff |gidx - (10*self_stock + 4.5)| < 5
            dm = msb.tile([128, 64], F32, tag="dm")
            nc.gpsimd.tensor_tensor(
                out=dm[0:MTS, :], in0=midxf[0:MTS, :],
                in1=gslo[0:MTS, g:g + 1].to_broadcast([MTS, 64]),
                op=ALU.subtract)
            nc.gpsimd.tensor_tensor(out=dm[0:MTS, :], in0=dm[0:MTS, :],
                                    in1=dm[0:MTS, :], op=ALU.mult)
            nc.vector.tensor_scalar(out=dm[0:MTS, :], in0=dm[0:MTS, :],
                                    scalar1=25.0, scalar2=None, op0=ALU.is_lt)
            nc.vector.scalar_tensor_tensor(out=mvalc[0:MTS, :], in0=dm[0:MTS, :],
                                           scalar=-1e30, in1=mvalc[0:MTS, :],
                                           op0=ALU.mult, op1=ALU.add)
            v8m = msb.tile([128, 8], F32, tag="v8m")
            nc.vector.max(out=v8m[0:MTS, :], in_=mvalc[0:MTS, :])
            pos8 = msb.tile([128, 8], U32, tag="pos8")
            nc.vector.max_index(out=pos8[0:MTS, :], in_max=v8m[0:MTS, :],
                                in_values=mvalc[0:MTS, :])
            pos5f = msb.tile([128, 5], F32, tag="pos5f")
            nc.vector.tensor_copy(pos5f[0:MTS, :], pos8[0:MTS, 0:5])
            eq = msb.tile([128, 5 * 64], F32, tag="eq")
            eq_v = eq[0:MTS, :].rearrange("p (k c) -> p k c", c=64)
            nc.vector.tensor_tensor(
                out=eq_v,
                in0=pos5f[0:MTS, :].rearrange("p k -> p k ()").to_broadcast([MTS, 5, 64]),
                in1=iota_f[0:MTS, :].rearrange("p c -> p () c").to_broadcast([MTS, 5, 64]),
                op=ALU.is_equal)
            nc.vector.tensor_tensor(
                out=eq_v, in0=eq_v,
                in1=midxf[0:MTS, :].rearrange("p c -> p () c").to_broadcast([MTS, 5, 64]),
                op=ALU.mult)
            g5f = msb.tile([128, 5], F32, tag="g5f")
            nc.vector.tensor_reduce(out=g5f[0:MTS, :], in_=eq_v,
                                    axis=mybir.AxisListType.X, op=ALU.add)
            g5u = msb.tile([128, 5], U32, tag="g5u")
            nc.vector.tensor_copy(g5u[0:MTS, :], g5f[0:MTS, :])
            # z rows (6 raw feats + lag_bias + pad) for the 5 winners
            zb = msb.tile([128, 5 * 8], F32, tag="zb")
            for r in range(K):
                nc.gpsimd.indirect_dma_start(
                    out=zb[0:MTS, r * 8:(r + 1) * 8], out_offset=None,
                    in_=d_xzb[:, :],
                    in_offset=IndirectOffsetOnAxis(ap=g5u[0:MTS, r:r + 1], axis=0))
            vb5 = msb.tile([128, 5], F32, tag="vb5")
            nc.vector.tensor_tensor(out=vb5[0:MTS, :], in0=v8m[0:MTS, 0:5],
                                    in1=zb[0:MTS, 6::8], op=ALU.add)
            # softmax over the 5 candidate scores
            mx = msb.tile([128, 1], F32, tag="mx")
            nc.vector.tensor_reduce(out=mx[0:MTS, :], in_=vb5[0:MTS, :],
                                    axis=mybir.AxisListType.X, op=ALU.max)
            nmx = msb.tile([128, 1], F32, tag="nmx")
            nc.vector.tensor_scalar(out=nmx[0:MTS, :], in0=mx[0:MTS, :],
                                    scalar1=-1.0, scalar2=None, op0=ALU.mult)
            e5 = msb.tile([128, 5], F32, tag="e5")
            nc.scalar.activation(e5[0:MTS, :], vb5[0:MTS, :], AF.Exp,
                                 bias=nmx[0:MTS, 0:1])
            ssum = msb.tile([128, 1], F32, tag="ssum")
            nc.vector.tensor_reduce(out=ssum[0:MTS, :], in_=e5[0:MTS, :],
                                    axis=mybir.AxisListType.X, op=ALU.add)
            rs = msb.tile([128, 1], F32, tag="rs")
            nc.vector.reciprocal(rs[0:MTS, :], ssum[0:MTS, :])
            w5 = msb.tile([128, 5], F32, tag="w5")
            nc.vector.tensor_scalar(out=w5[0:MTS, :], in0=e5[0:MTS, :],
                                    scalar1=rs[0:MTS, 0:1], scalar2=None, op0=ALU.mult)
            # z_agg = sum_r w_r * z_r ; feat = [z_agg, z_0]
            wz = msb.tile([128, 5 * 6], F32, tag="wz")
            zview = zb[0:MTS, :].rearrange("p (r w) -> p r w", w=8)[:, :, 0:6]
            nc.vector.tensor_tensor(
                out=wz[0:MTS, :].rearrange("p (r f) -> p r f", f=6),
                in0=zview,
                in1=w5[0:MTS, :].rearrange("p r -> p r ()").to_broadcast([MTS, 5, 6]),
                op=ALU.mult)
            feat = msb.tile([128, 2 * F], F32, tag="feat")
            nc.vector.tensor_reduce(
                out=feat[0:MTS, 0:6],
                in_=wz[0:MTS, :].rearrange("p (r f) -> p f r", f=6),
                axis=mybir.AxisListType.X, op=ALU.add)
            nc.vector.tensor_copy(feat[0:MTS, 6:12], zb[0:MTS, 0:6])
            # MLP head (tiny fp32 matmuls; PSUM tags reused down the chain)
            pft = mps.tile([128, 128], F32, tag="mlpA", space="PSUM")
            nc.tensor.transpose(out=pft[0:2 * F, 0:128], in_=feat[:, :],
                                identity=ident[:])
            featT = msb.tile([2 * F, 128], F32, tag="featT")
            nc.scalar.activation(featT[:], pft[0:2 * F, 0:128], AF.Copy)
            ph1 = mps.tile([128, 128], F32, tag="mlpB", space="PSUM")
            nc.tensor.matmul(out=ph1[0:64, :], lhsT=w1t[:], rhs=featT[:],
                             start=True, stop=True)
            h1 = msb.tile([64, 128], F32, tag="h1")
            nc.scalar.activation(h1[:], ph1[0:64, :], AF.Relu, bias=b1c[:, 0:1])
            ph2 = mps.tile([128, 128], F32, tag="mlpA", space="PSUM")
            nc.tensor.matmul(out=ph2[0:32, :], lhsT=w2t[:], rhs=h1[:],
                             start=True, stop=True)
            h2 = msb.tile([32, 128], F32, tag="h2")
            nc.scalar.activation(h2[:], ph2[0:32, :], AF.Relu, bias=b2c[:, 0:1])
            py_ = mps.tile([128, 128], F32, tag="mlpB", space="PSUM")
            nc.tensor.matmul(out=py_[0:1, :], lhsT=w3t[:], rhs=h2[:],
                             start=True, stop=True)
            yrow = msb.tile([1, 128], F32, tag="yrow")
            nc.scalar.activation(yrow[:], py_[0:1, :], AF.Identity, bias=b3c[0:1, 0:1])
            pyt = mps.tile([128, 128], F32, tag="mlpA", space="PSUM")
            nc.tensor.transpose(out=pyt[:, 0:1], in_=yrow[0:1, :],
                                identity=ident[0:1, 0:1])
            ycol = msb.tile([128, 1], F32, tag="ycol")
            nc.vector.tensor_copy(ycol[:], pyt[:, 0:1])
            nc.gpsimd.indirect_dma_start(
                out=d_y[:, :],
                out_offset=IndirectOffsetOnAxis(ap=orows[0:MTS, g:g + 1], axis=0),
                in_=ycol[0:MTS, :], in_offset=None)

        for g in range(NG):
            for T8 in range(GT):
                T = GT * g + T8
                # per-tile transposed queries (gathers pipeline on Pool).
                # Group 0's targets reference only A-rows by construction;
                # later tiles mix A and B rows, filled by two bounds-checked
                # gathers (out-of-bounds offsets are skipped silently).
                qrow = gsb.tile([128, 128], F32, tag="qrow")
                if g == 0:
                    nc.gpsimd.indirect_dma_start(
                        out=qrow[:], out_offset=None, in_=d_qbaA[:, :],
                        in_offset=IndirectOffsetOnAxis(ap=trowsA[:, T:T + 1], axis=0))
                else:
                    nc.gpsimd.indirect_dma_start(
                        out=qrow[:], out_offset=None, in_=d_qbaA[:, :],
                        in_offset=IndirectOffsetOnAxis(ap=trowsA[:, T:T + 1], axis=0),
                        bounds_check=ND * aw - 1, oob_is_err=False)
                    nc.gpsimd.indirect_dma_start(
                        out=qrow[:], out_offset=None, in_=d_qbaB[:, :],
                        in_offset=IndirectOffsetOnAxis(ap=trowsB[:, T:T + 1], axis=0),
                        bounds_check=ND * (SS - aw) - 1, oob_is_err=False)
                if g == 0 and T8 == GT - 1:
                    # all of group 0's gathers are issued; fire the late q
                    # exchange (it completes well before group 1 needs it)
                    nc.gpsimd.collective_compute(
                        "AllGather", ALU.bypass, replica_groups=groups,
                        ins=[d_qblB.ap().opt()], outs=[d_qbaB.ap().opt()])
                pt = gps_.tile([128, 128], F32, tag="pt", space="PSUM")
                nc.tensor.transpose(out=pt[:], in_=qrow[:], identity=ident[:])
                qT = qT_all[:, T * 128:(T + 1) * 128]
                nc.scalar.activation(qT, pt[:], AF.Copy)
                ssb = ssb_p.tile([128, COLS], F32, tag="ssb")
                for c0 in range(0, COLS, SCH):
                    c1 = min(COLS, c0 + SCH)
                    ps = sps.tile([128, SCH], F32, tag="sc", space="PSUM")
                    for b0 in range(0, c1 - c0, 512):
                        b1 = min(c1 - c0, b0 + 512)
                        nc.tensor.matmul(out=ps[:, b0:b1], lhsT=qT,
                                         rhs=keysT[:, c0 + b0:c0 + b1],
                                         start=True, stop=True)
                    nc.scalar.activation(ssb[:, c0:c1], ps[:, 0:c1 - c0], AF.Copy)
                nc.vector.max(out=v8[:, T * 8:(T + 1) * 8], in_=ssb[:])
                nc.vector.max_index(out=i8[:, T * 8:(T + 1) * 8],
                                    in_max=v8[:, T * 8:(T + 1) * 8], in_values=ssb[:])
            # merge previous group's exchanged candidates (its AllGather has
            # had a full group of score tiles to complete in)
            if g >= 1:
                merge_group(g - 1)
            # pack + exchange this group's candidates UNMASKED (the self
            # filter runs on the merge side, so the exchange depends only on
            # the scans and fires immediately)
            gsl = slice(g * GT * 8, (g + 1) * GT * 8)
            nc.sync.dma_start(
                out=cl_v[g][:, :, 0:8],
                in_=v8[:, gsl].rearrange("p (T w) -> p T w", w=8).bitcast(U32))
            nc.sync.dma_start(
                out=cl_v[g][:, :, 8:16],
                in_=i8[:, gsl].rearrange("p (T w) -> p T w", w=8))
            nc.gpsimd.collective_compute(
                "AllGather", ALU.bypass, replica_groups=groups,
                ins=[d_clg[g].ap().opt()], outs=[d_cag[g].ap().opt()])
        merge_group(NG - 1)

        mps.release()
        msb.release()
        sps.release()
        ssb_p.release()
        gps_.release()
        gsb.release()
        big.release()
        cpool.release()

    nc.compile()
    return nc


_CACHED_NC = {}
_BIGROW = 10_000_000


def _assignment(tix):
    """Per-owner target->(group, slot) assignment: each owner's 375 targets
    sorted by the referenced stock's local index lam = tix % SS; group g gets
    the g-th 125 of that order (so group 0 needs only low-lam q rows), and
    the A/B split threshold aw covers every group-0 lam."""
    asg = np.zeros((ND, NG, MTS), np.int64)
    aw = 126
    for d in range(ND):
        ts = np.arange(d * SS, (d + 1) * SS)
        order = np.argsort(tix[ts] % SS, kind="stable")
        for gg in range(NG):
            asg[d, gg] = ts[order[gg * MTS:(gg + 1) * MTS]]
        aw = max(aw, int((tix[asg[d, 0]] % SS).max()) + 1)
    assert aw <= 250, f"group-0 lam spread too wide: {aw}"
    return asg, aw


def _prep_inputs(X_scaled, X_raw, target_idx, lstm_Wih, lstm_Whh, lstm_bih,
                 lstm_bhh, ln_g, ln_b, WQ, WK, log_temp, lag_bias,
                 W1, b1, W2, b2, W3, b3):
    f32 = np.float32
    assert np.all(np.asarray(ln_b) == 0.0), "kernel assumes ln_b == 0"
    tix = np.asarray(target_idx).astype(np.int64)
    bias = (np.asarray(lstm_bih) + np.asarray(lstm_bhh)).astype(f32)
    gperm = np.r_[0:N, N:2 * N, 3 * N:4 * N, 2 * N:3 * N]    # [i, f, o, g]
    g_ln = np.asarray(ln_g).astype(f32)
    wq_f = (np.asarray(WQ) * g_ln[None, :]).astype(f32)
    wk_f = (np.asarray(WK) * g_ln[None, :]).astype(f32)
    uq = np.asarray(WQ) @ g_ln
    uk = np.asarray(WK) @ g_ln
    inv_temp = np.asarray(
        1.0 / np.clip(np.exp(np.asarray(log_temp, np.float64)), 0.1, np.sqrt(N)),
        f32).reshape(1, 1)

    # XZB table: flat (s,l) -> [6 raw feats at lag_pos, lag_bias, 0]
    Xr = np.asarray(X_raw)[0].astype(f32)                    # [S, L, F]
    lb = np.asarray(lag_bias).astype(f32)
    xzb = np.zeros((S * LMAX, 8), f32)
    lagpos = np.clip(L - 1 - (LMAX - np.arange(LMAX)), 0, L - 1)
    xzb[:, 0:6] = Xr[:, lagpos, :].reshape(S * LMAX, 6)
    xzb[:, 6] = np.tile(lb, S)

    # lam-sorted position assignment; target asg[d, g, o] sits at position
    # g*1024 + d*128 + o (tile T = 8g + d, row o)
    asg, aw = _assignment(tix)
    bw = SS - aw
    trowsA = np.full((128, NT), _BIGROW, np.uint32)
    trowsB = np.full((128, NT), _BIGROW, np.uint32)
    trowsA[:, 0:GT] = 0  # group-0 pad rows still gather a valid row
    gslo = np.full((128, NG, ND), -1e9, f32)
    orows = np.zeros((128, NG, ND), np.uint32)
    mrows = np.zeros((128, NG * ND, ND), np.uint32)
    for d in range(ND):
        for gg in range(NG):
            for o in range(MTS):
                t = asg[d, gg, o]
                sig = int(tix[t])
                ow, lam = divmod(sig, SS)
                T = GT * gg + d
                if lam < aw:
                    trowsA[o, T] = ow * aw + lam
                    trowsB[o, T] = _BIGROW
                else:
                    assert gg > 0
                    trowsA[o, T] = _BIGROW
                    trowsB[o, T] = ow * bw + (lam - aw)
                gslo[o, gg, d] = sig * LMAX + 4.5
                orows[o, gg, d] = t
            for s in range(ND):
                mrows[:MTS, gg * ND + s, d] = s * GPOS + d * 128 + np.arange(MTS)

    Xs = np.asarray(X_scaled)[0].astype(f32)                 # [S, L, F]
    common = dict(
        wih_t=np.ascontiguousarray(np.vstack([
            np.asarray(lstm_Wih).astype(f32).T, bias[None, :]])[:, gperm]),
        whh_t=np.ascontiguousarray(np.asarray(lstm_Whh).astype(f32).T[:, gperm]),
        wq_t=np.ascontiguousarray(wq_f.T), wk_t=np.ascontiguousarray(wk_f.T),
        negu_q=np.ascontiguousarray((-uq.astype(f32) / N).reshape(1, N)),
        negu_k=np.ascontiguousarray((-uk.astype(f32) / N).reshape(1, N)),
        invt=inv_temp, xzb=xzb, tgtrowsA=trowsA, tgtrowsB=trowsB,
        w1_t=np.ascontiguousarray(np.asarray(W1).astype(f32).T),
        w2_t=np.ascontiguousarray(np.asarray(W2).astype(f32).T),
        w3_t=np.ascontiguousarray(np.asarray(W3).astype(f32).T),
        b1c=np.asarray(b1).astype(f32).reshape(64, 1),
        b2c=np.asarray(b2).astype(f32).reshape(32, 1),
        b3c=np.asarray(b3).astype(f32).reshape(1, 1),
    )
    in_maps = []
    for d in range(ND):
        # time-major xt: [F+1, L*SS], column = t*SS + s
        xtv = np.ascontiguousarray(np.vstack([
            Xs[d * SS:(d + 1) * SS].transpose(2, 1, 0).reshape(F, L * SS),
            np.ones((1, L * SS), f32)]))
        in_maps.append(dict(
            common, xt=xtv,
            gslo=np.ascontiguousarray(gslo[:, :, d]),
            mrows=np.ascontiguousarray(mrows[:, :, d]),
            orows=np.ascontiguousarray(orows[:, :, d]),
        ))
    return in_maps


def kernel(**inputs):
    tix = np.asarray(inputs["target_idx"]).astype(np.int64)
    _, aw = _assignment(tix)
    if aw not in _CACHED_NC:
        _CACHED_NC[aw] = build_program(aw)
    nc = _CACHED_NC[aw]
    in_maps = _prep_inputs(**inputs)
    res = run_bass_kernel_spmd(nc, in_maps, core_ids=list(range(ND)))
    y = np.zeros(S, np.float32)
    for d in range(ND):
        y[d * SS:(d + 1) * SS] = res.results[d]["y"][d * SS:(d + 1) * SS, 0]
    return y


# revision 59
# speedup vs baseline: 1.0329x; 1.0329x over previous
"""Trainium2 Bass kernel for nn_DeltaLag (LSTM encoder + lagged cross-attention
top-k + MLP head), distributed over 8 NeuronCores.

Sharding: stocks are split 375/core (LSTM + keys local to each core); every
core computes the score block [3072 padded target positions x 3750 local
(stock,lag) columns] in fp32, takes its local top-8 per target, and candidate
(value, index) pairs are exchanged with three pipelined AllToAlls (one per
8-tile position group) so the exchange and the per-group merge overlap the
next group's score computation. Each core merges + finishes its own 375
targets (z-gather + softmax + MLP).

Position layout: target t (owner d = t//375, i = t%375, g = i//125, o=i%125)
lives at position g*1024 + d*128 + o, i.e. tile T = 8g + d, row o. A group's
AllGather over rows [g*1024, (g+1)*1024) delivers core d's targets'
candidates from every source core at rows s*1024 + d*128 + o.

The compiled program is identical on all 8 cores (SPMD); everything
device-specific (shards, self-column ids, gather indices) is passed as input
tensors. All matmuls run in true fp32 (fp32r measured at ~1e-3 relative error
on this hardware, which would flip top-k selections).
"""

import sys

sys.path.insert(0, "/opt/trn_rl_repo")

import numpy as np

import concourse.bacc as bacc
import concourse.mybir as mybir
import concourse.tile as tile
from concourse.bass import IndirectOffsetOnAxis
from concourse.bass_utils import run_bass_kernel_spmd
from concourse.masks import make_identity

F32 = mybir.dt.float32
U32 = mybir.dt.uint32
U16 = mybir.dt.uint16
AF = mybir.ActivationFunctionType
ALU = mybir.AluOpType

S, F, N, L, LMAX, K = 3000, 6, 128, 40, 10, 5
ND = 8                      # cores
SS = S // ND                # stocks per core
COLS = SS * LMAX            # score columns per core
NG = 3                      # candidate-exchange groups
GPOS = ND * 128             # positions per group (1024)
NPOS = NG * GPOS            # padded target count (3072)
NT = NPOS // 128            # target tiles (24)
GT = NT // NG               # tiles per group (8)
MTS = 125                   # used rows per (group, owner) slot
SCH = 1024                  # score-tile PSUM chunk width
XCH = 8                     # xt DMA chunks (5 timesteps each)
CW16 = 16                   # u32 words per exchanged candidate row (8 v + 8 idx)


def build_program(aw):
    """aw: per-core stock-index threshold splitting the q AllGather into an
    early small exchange (rows [0,aw), enough for group-0's targets) and a
    late one hidden under group-0's score scans."""
    assert 125 <= aw <= 250
    bw = SS - aw
    nc = bacc.Bacc("TRN2", target_bir_lowering=False, debug=False,
                   enable_asserts=True, num_devices=ND)

    # ---- I/O ----
    d_xt = nc.dram_tensor("xt", [F + 1, L * SS], F32, kind="ExternalInput")
    d_wih = nc.dram_tensor("wih_t", [F + 1, 4 * N], F32, kind="ExternalInput")
    d_whh = nc.dram_tensor("whh_t", [N, 4 * N], F32, kind="ExternalInput")
    d_wqt = nc.dram_tensor("wq_t", [N, N], F32, kind="ExternalInput")
    d_wkt = nc.dram_tensor("wk_t", [N, N], F32, kind="ExternalInput")
    d_nuq = nc.dram_tensor("negu_q", [1, N], F32, kind="ExternalInput")
    d_nuk = nc.dram_tensor("negu_k", [1, N], F32, kind="ExternalInput")
    d_invt = nc.dram_tensor("invt", [1, 1], F32, kind="ExternalInput")
    d_gslo = nc.dram_tensor("gslo", [128, NG], F32, kind="ExternalInput")
    d_trowsA = nc.dram_tensor("tgtrowsA", [128, NT], U32, kind="ExternalInput")
    d_trowsB = nc.dram_tensor("tgtrowsB", [128, NT], U32, kind="ExternalInput")
    d_xzb = nc.dram_tensor("xzb", [S * LMAX, 8], F32, kind="ExternalInput")
    d_mrows = nc.dram_tensor("mrows", [128, NG * ND], U32, kind="ExternalInput")
    d_orows = nc.dram_tensor("orows", [128, NG], U32, kind="ExternalInput")
    d_w1t = nc.dram_tensor("w1_t", [2 * F, 64], F32, kind="ExternalInput")
    d_w2t = nc.dram_tensor("w2_t", [64, 32], F32, kind="ExternalInput")
    d_w3t = nc.dram_tensor("w3_t", [32, 1], F32, kind="ExternalInput")
    d_b1 = nc.dram_tensor("b1c", [64, 1], F32, kind="ExternalInput")
    d_b2 = nc.dram_tensor("b2c", [32, 1], F32, kind="ExternalInput")
    d_b3 = nc.dram_tensor("b3c", [1, 1], F32, kind="ExternalInput")

    d_y = nc.dram_tensor("y", [S, 1], F32, kind="ExternalOutput")

    # d_qblB carries one extra fence row (bw): written only after group 0's
    # q gathers are issued, so the B exchange cannot jump ahead of them on
    # the Pool engine's ready queue and delay group 0's scores.
    d_qblA = nc.dram_tensor("qb_localA", [aw, N], F32)
    d_qblB = nc.dram_tensor("qb_localB", [bw + 1, N], F32)
    d_qbaA = nc.dram_tensor("qb_allA", [ND * aw, N], F32, addr_space="Shared")
    d_qbaB = nc.dram_tensor("qb_allB", [ND * (bw + 1), N], F32,
                            addr_space="Shared")
    d_clg = [nc.dram_tensor(f"cand_local{g}", [GPOS, CW16], U32)
             for g in range(NG)]
    d_cag = [nc.dram_tensor(f"cand_all{g}", [ND * GPOS, CW16], U32,
                            addr_space="Shared") for g in range(NG)]

    groups = [list(range(ND))]

    with tile.TileContext(nc) as tc:
        cpool = tc.alloc_tile_pool(name="const", bufs=1)
        big = tc.alloc_tile_pool(name="big", bufs=1)

        # ---- constants / params to SBUF ----
        ident = cpool.tile([128, 128], F32)
        make_identity(nc, ident[:])
        ones1 = cpool.tile([1, 128], F32)
        nc.vector.memset(ones1[:], 1.0)
        onesf = cpool.tile([128, 128], F32)
        nc.vector.memset(onesf[:], 1.0)

        def load(pool, dram, shape, dtype=F32):
            t = pool.tile(shape, dtype, tag=f"ld_{dram.name}")
            nc.sync.dma_start(out=t[:], in_=dram[:, :])
            return t

        # LSTM weights and the ppre inputs first (the SP DMA queue is
        # in-order; PE's first scheduled ops are the ppre matmuls and the
        # first LSTM step, which need these plus only the first xt chunk)
        wih = load(cpool, d_wih, [F + 1, 4 * N])
        whh = load(cpool, d_whh, [N, 4 * N])
        nuq = load(cpool, d_nuq, [1, N])
        nuk = load(cpool, d_nuk, [1, N])
        # time-major xt arrives in XCH separate chunk tiles so the LSTM's
        # step-t matmul depends only on its own chunk's DMA
        TPC = L // XCH
        CW = TPC * SS
        xts = []
        for c in range(XCH):
            xtc = big.tile([F + 1, CW], F32, tag=f"xt{c}")
            nc.sync.dma_start(out=xtc[:], in_=d_xt[:, c * CW:(c + 1) * CW])
            xts.append(xtc)
        wqt = load(cpool, d_wqt, [N, N])
        wkt = load(cpool, d_wkt, [N, N])
        invt = load(cpool, d_invt, [1, 1])
        trowsA = load(cpool, d_trowsA, [128, NT], U32)
        trowsB = load(cpool, d_trowsB, [128, NT], U32)
        mrows = load(cpool, d_mrows, [128, NG * ND], U32)
        orows = load(cpool, d_orows, [128, NG], U32)
        w1t = load(cpool, d_w1t, [2 * F, 64])
        w2t = load(cpool, d_w2t, [64, 32])
        w3t = load(cpool, d_w3t, [32, 1])
        b1c = load(cpool, d_b1, [64, 1])
        b2c = load(cpool, d_b2, [32, 1])
        b3c = load(cpool, d_b3, [1, 1])

        gslo = load(cpool, d_gslo, [128, NG])
        invtb = cpool.tile([128, 1], F32)
        nc.gpsimd.partition_broadcast(invtb[:], invt[:], channels=128)

        iota_u = cpool.tile([128, 64], U32)
        nc.gpsimd.iota(iota_u[:], pattern=[[1, 64]], base=0, channel_multiplier=0)
        iota_f = cpool.tile([128, 64], F32)
        nc.vector.tensor_copy(iota_f[:], iota_u[:])
        base_u = cpool.tile([128, 64], U32)
        nc.gpsimd.iota(base_u[:], pattern=[[COLS, 8], [0, 8]], base=0,
                       channel_multiplier=0)
        base_f = cpool.tile([128, 64], F32)
        nc.vector.tensor_copy(base_f[:], base_u[:])

        # rank-1 LN-fold correction matrices: rows n, cols p -> -u[p]/128
        with tc.tile_pool(name="ppre", bufs=1, space="PSUM") as ppre:
            uqo = cpool.tile([128, 128], F32)
            uko = cpool.tile([128, 128], F32)
            pq = ppre.tile([128, 128], F32, space="PSUM")
            nc.tensor.matmul(out=pq[:], lhsT=ones1[:], rhs=nuq[:], start=True, stop=True)
            nc.scalar.activation(uqo[:], pq[:], AF.Copy)
            pk = ppre.tile([128, 128], F32, space="PSUM")
            nc.tensor.matmul(out=pk[:], lhsT=ones1[:], rhs=nuk[:], start=True, stop=True)
            nc.scalar.activation(uko[:], pk[:], AF.Copy)

        # ---- Phase 1: LSTM over the 375 local stocks ----
        # h,c layout [n=128, s]; last-10 hidden states land in hsave[n, s*10+k].
        # Gate columns in wih/whh are host-permuted to [i, f, o, g]; the bias
        # is folded into the xproj matmul via xt's constant-1 row.
        hsave = big.tile([128, COLS], F32)
        with tc.tile_pool(name="lstm_sb", bufs=2) as lsb, \
             tc.tile_pool(name="lstm_c", bufs=2) as lcp, \
             tc.tile_pool(name="lstm_ps", bufs=2, space="PSUM") as lps:
            h0 = lsb.tile([128, SS], F32, tag="h0")
            nc.vector.memset(h0[:], 0.0)
            c_prev = lcp.tile([128, SS], F32, tag="c")
            nc.vector.memset(c_prev[:], 0.0)
            h_prev = h0[:]
            for t in range(L):
                xs = xts[t // TPC][:, (t % TPC) * SS:(t % TPC + 1) * SS]
                # all four x-projections first: they don't depend on h_{t-1},
                # so the PE has work while the gate elementwise chain finishes
                gps = []
                for g in range(4):
                    psg = lps.tile([128, SS], F32, tag=f"g{g}", space="PSUM")
                    nc.tensor.matmul(out=psg[:], lhsT=wih[:, g * N:(g + 1) * N],
                                     rhs=xs, start=True, stop=False)
                    gps.append(psg)
                for g in range(4):
                    nc.tensor.matmul(out=gps[g][:], lhsT=whh[:, g * N:(g + 1) * N],
                                     rhs=h_prev, start=False, stop=True)
                si = lsb.tile([128, SS], F32, tag="si")
                nc.scalar.activation(si[:], gps[0][:], AF.Sigmoid)
                sf = lsb.tile([128, SS], F32, tag="sf")
                nc.scalar.activation(sf[:], gps[1][:], AF.Sigmoid)
                so_t = lsb.tile([128, SS], F32, tag="so")
                nc.scalar.activation(so_t[:], gps[2][:], AF.Sigmoid)
                tg = lsb.tile([128, SS], F32, tag="tg")
                nc.scalar.activation(tg[:], gps[3][:], AF.Tanh)
                si, sf, so = si[:], sf[:], so_t[:]
                t1 = lsb.tile([128, SS], F32, tag="t1")
                nc.vector.tensor_tensor(out=t1[:], in0=si, in1=tg[:], op=ALU.mult)
                c2 = lsb.tile([128, SS], F32, tag="c2")
                nc.gpsimd.tensor_tensor(out=c2[:], in0=sf, in1=c_prev[:], op=ALU.mult)
                c_new = lcp.tile([128, SS], F32, tag="c")
                nc.vector.tensor_tensor(out=c_new[:], in0=c2[:], in1=t1[:], op=ALU.add)
                th = lsb.tile([128, SS], F32, tag="th")
                nc.scalar.activation(th[:], c_new[:], AF.Tanh)
                if t >= L - LMAX:
                    h_out = hsave[:, (t - (L - LMAX))::LMAX]
                    nc.gpsimd.tensor_tensor(out=h_out, in0=so, in1=th[:], op=ALU.mult)
                    h_prev = h_out
                else:
                    hn = lsb.tile([128, SS], F32, tag="hn")
                    nc.gpsimd.tensor_tensor(out=hn[:], in0=so, in1=th[:], op=ALU.mult)
                    h_prev = hn[:]
                c_prev = c_new

        # ---- Phase 2: queries first, so the AllGather overlaps the keys ----
        with tc.tile_pool(name="q_sb", bufs=2) as qsb, \
             tc.tile_pool(name="q_ps", bufs=2, space="PSUM") as qps:
            h39 = hsave[:, (LMAX - 1)::LMAX]
            pyq = qps.tile([128, SS], F32, tag="yq", space="PSUM")
            nc.tensor.matmul(out=pyq[:], lhsT=wqt[:], rhs=h39, start=True, stop=False)
            nc.tensor.matmul(out=pyq[:], lhsT=uqo[:], rhs=h39, start=False, stop=True)
            yq = qsb.tile([128, SS], F32, tag="yq_sb")
            nc.scalar.activation(yq[:], pyq[:], AF.Copy)
            y2q = qsb.tile([128, SS], F32, tag="y2q")
            nc.scalar.activation(y2q[:], pyq[:], AF.Square)
            psq = qps.tile([128, SS], F32, tag="sq", space="PSUM")
            nc.tensor.matmul(out=psq[:], lhsT=onesf[:], rhs=y2q[:], start=True, stop=True)
            sq = qsb.tile([128, SS], F32, tag="sqq")
            nc.scalar.sqrt(sq[:], psq[:])
            ri = qsb.tile([128, SS], F32, tag="riq")
            nc.vector.reciprocal(ri[:], sq[:])
            qn1 = qsb.tile([128, SS], F32, tag="qn1")
            nc.vector.tensor_tensor(out=qn1[:], in0=yq[:], in1=ri[:], op=ALU.mult)
            qn = qsb.tile([128, SS], F32, tag="qn")
            nc.vector.tensor_scalar(out=qn[:], in0=qn1[:], scalar1=invtb[:, 0:1],
                                    scalar2=None, op0=ALU.mult)
            for j in range(3):
                pt = qps.tile([128, 128], F32, tag="qt", space="PSUM")
                nc.tensor.transpose(out=pt[0:MTS, :], in_=qn[:, j * MTS:(j + 1) * MTS],
                                    identity=ident[:])
                qrow_sb = qsb.tile([128, 128], F32, tag="qrow_sb")
                nc.scalar.activation(qrow_sb[0:MTS, :], pt[0:MTS, :], AF.Copy)
                # split the q rows at local stock index `aw` between the
                # early (A) and late (B) exchange tensors
                lo, hi = j * MTS, j * MTS + MTS
                if hi <= aw:
                    nc.sync.dma_start(out=d_qblA[lo:hi, :], in_=qrow_sb[0:MTS, :])
                elif lo >= aw:
                    nc.sync.dma_start(out=d_qblB[lo - aw:hi - aw, :],
                                      in_=qrow_sb[0:MTS, :])
                else:
                    na = aw - lo
                    nc.sync.dma_start(out=d_qblA[lo:aw, :], in_=qrow_sb[0:na, :])
                    nc.sync.dma_start(out=d_qblB[0:hi - aw, :],
                                      in_=qrow_sb[na:MTS, :])
                if hi >= aw and lo < aw:
                    # all A rows written: fire the early exchange now
                    nc.gpsimd.collective_compute(
                        "AllGather", ALU.bypass, replica_groups=groups,
                        ins=[d_qblA.ap().opt()], outs=[d_qbaA.ap().opt()])

        # ---- Phase 3: keys (LN+l2norm folded into matmuls; sigma cancels) ----
        keysT = big.tile([128, COLS], F32)
        with tc.tile_pool(name="key_sb", bufs=3) as ksb, \
             tc.tile_pool(name="key_ps", bufs=2, space="PSUM") as kps:
            ysb = big.tile([128, COLS], F32)
            CH = 512
            chunks = [(c0, min(COLS, c0 + CH)) for c0 in range(0, COLS, CH)]
            sqs = []
            for c0, c1 in chunks:
                w = c1 - c0
                py = kps.tile([128, CH], F32, tag="y", space="PSUM")
                nc.tensor.matmul(out=py[:, :w], lhsT=wkt[:],
                                 rhs=hsave[:, c0:c1], start=True, stop=False)
                nc.tensor.matmul(out=py[:, :w], lhsT=uko[:],
                                 rhs=hsave[:, c0:c1], start=False, stop=True)
                nc.scalar.activation(ysb[:, c0:c1], py[:, :w], AF.Copy)
                y2 = ksb.tile([128, CH], F32, tag="y2")
                nc.scalar.activation(y2[:, :w], py[:, :w], AF.Square)
                psq = kps.tile([128, CH], F32, tag="s", space="PSUM")
                nc.tensor.matmul(out=psq[:, :w], lhsT=onesf[:],
                                 rhs=y2[:, :w], start=True, stop=True)
                sq = ksb.tile([128, CH], F32, tag="sq")
                nc.scalar.sqrt(sq[:, :w], psq[:, :w])
                sqs.append((sq, c0, c1))
            for sq, c0, c1 in sqs:
                w = c1 - c0
                ri = ksb.tile([128, CH], F32, tag="ri")
                nc.vector.reciprocal(ri[:, :w], sq[:, :w])
                nc.vector.tensor_tensor(out=keysT[:, c0:c1], in0=ysb[:, c0:c1],
                                        in1=ri[:, :w], op=ALU.mult)

        # ---- Phase 4+5+6: per group: qT gather/transpose + scores + top-8
        # (per tile), candidate AllGather exchange, and the merge of the
        # PREVIOUS group's exchanged candidates ----
        qT_all = big.tile([128, NPOS], F32)
        v8 = big.tile([128, NT * 8], F32)
        i8 = big.tile([128, NT * 8], U32)
        cl_v = [d_clg[g].ap().rearrange("(T p) w -> p T w", p=128)
                for g in range(NG)]

        gsb = tc.alloc_tile_pool(name="qg_sb", bufs=3)
        gps_ = tc.alloc_tile_pool(name="qg_ps", bufs=2, space="PSUM")
        ssb_p = tc.alloc_tile_pool(name="sc_sb", bufs=2)
        sps = tc.alloc_tile_pool(name="sc_ps", bufs=2, space="PSUM")
        msb = tc.alloc_tile_pool(name="m_sb", bufs=2)
        mps = tc.alloc_tile_pool(name="m_ps", bufs=1, space="PSUM")

        def merge_group(g):
            # candidates for my 125 targets of group g, from all 8 cores
            mv = msb.tile([128, ND * CW16], U32, tag="mv")
            for s in range(ND):
                nc.gpsimd.indirect_dma_start(
                    out=mv[0:MTS, s * CW16:(s + 1) * CW16], out_offset=None,
                    in_=d_cag[g][:, :],
                    in_offset=IndirectOffsetOnAxis(
                        ap=mrows[0:MTS, g * ND + s:g * ND + s + 1], axis=0))
            mvals = mv[0:MTS, :].bitcast(F32).rearrange(
                "p (d w) -> p d w", w=CW16)[:, :, 0:8]
            midx = mv[0:MTS, :].rearrange("p (d w) -> p d w", w=CW16)[:, :, 8:16]
            # leading copies + self-filter arithmetic on gpsimd so the DVE's
            # scan stream is disturbed as little as possible
            mvalc = msb.tile([128, 64], F32, tag="mvalc")
            nc.gpsimd.tensor_copy(mvalc[0:MTS, :], mvals)
            # global flat candidate index = owner*COLS + local
            midxf = msb.tile([128, 64], F32, tag="midxf")
            nc.gpsimd.tensor_copy(midxf[0:MTS, :], midx)
            nc.gpsimd.tensor_tensor(out=midxf[0:MTS, :], in0=midxf[0:MTS, :],
                                    in1=base_f[0:MTS, :], op=ALU.add)
            # self-stock filter (candidates were exchanged unmasked):
            # self iff |gidx - (10*self_stock + 4.5)| < 5
            dm = msb.tile([128, 64], F32, tag="dm")
            nc.gpsimd.tensor_tensor(
                out=dm[0:MTS, :], in0=midxf[0:MTS, :],
                in1=gslo[0:MTS, g:g + 1].to_broadcast([MTS, 64]),
                op=ALU.subtract)
            nc.gpsimd.tensor_tensor(out=dm[0:MTS, :], in0=dm[0:MTS, :],
                                    in1=dm[0:MTS, :], op=ALU.mult)
            nc.vector.tensor_scalar(out=dm[0:MTS, :], in0=dm[0:MTS, :],
                                    scalar1=25.0, scalar2=None, op0=ALU.is_lt)
            nc.vector.scalar_tensor_tensor(out=mvalc[0:MTS, :], in0=dm[0:MTS, :],
                                           scalar=-1e30, in1=mvalc[0:MTS, :],
                                           op0=ALU.mult, op1=ALU.add)
            v8m = msb.tile([128, 8], F32, tag="v8m")
            nc.vector.max(out=v8m[0:MTS, :], in_=mvalc[0:MTS, :])
            pos8 = msb.tile([128, 8], U32, tag="pos8")
            nc.vector.max_index(out=pos8[0:MTS, :], in_max=v8m[0:MTS, :],
                                in_values=mvalc[0:MTS, :])
            pos5f = msb.tile([128, 5], F32, tag="pos5f")
            nc.vector.tensor_copy(pos5f[0:MTS, :], pos8[0:MTS, 0:5])
            eq = msb.tile([128, 5 * 64], F32, tag="eq")
            eq_v = eq[0:MTS, :].rearrange("p (k c) -> p k c", c=64)
            nc.vector.tensor_tensor(
                out=eq_v,
                in0=pos5f[0:MTS, :].rearrange("p k -> p k ()").to_broadcast([MTS, 5, 64]),
                in1=iota_f[0:MTS, :].rearrange("p c -> p () c").to_broadcast([MTS, 5, 64]),
                op=ALU.is_equal)
            nc.vector.tensor_tensor(
                out=eq_v, in0=eq_v,
                in1=midxf[0:MTS, :].rearrange("p c -> p () c").to_broadcast([MTS, 5, 64]),
                op=ALU.mult)
            g5f = msb.tile([128, 5], F32, tag="g5f")
            nc.vector.tensor_reduce(out=g5f[0:MTS, :], in_=eq_v,
                                    axis=mybir.AxisListType.X, op=ALU.add)
            g5u = msb.tile([128, 5], U32, tag="g5u")
            nc.vector.tensor_copy(g5u[0:MTS, :], g5f[0:MTS, :])
            # z rows (6 raw feats + lag_bias + pad) for the 5 winners
            zb = msb.tile([128, 5 * 8], F32, tag="zb")
            for r in range(K):
                nc.gpsimd.indirect_dma_start(
                    out=zb[0:MTS, r * 8:(r + 1) * 8], out_offset=None,
                    in_=d_xzb[:, :],
                    in_offset=IndirectOffsetOnAxis(ap=g5u[0:MTS, r:r + 1], axis=0))
            vb5 = msb.tile([128, 5], F32, tag="vb5")
            nc.vector.tensor_tensor(out=vb5[0:MTS, :], in0=v8m[0:MTS, 0:5],
                                    in1=zb[0:MTS, 6::8], op=ALU.add)
            # softmax over the 5 candidate scores
            mx = msb.tile([128, 1], F32, tag="mx")
            nc.vector.tensor_reduce(out=mx[0:MTS, :], in_=vb5[0:MTS, :],
                                    axis=mybir.AxisListType.X, op=ALU.max)
            nmx = msb.tile([128, 1], F32, tag="nmx")
            nc.vector.tensor_scalar(out=nmx[0:MTS, :], in0=mx[0:MTS, :],
                                    scalar1=-1.0, scalar2=None, op0=ALU.mult)
            e5 = msb.tile([128, 5], F32, tag="e5")
            nc.scalar.activation(e5[0:MTS, :], vb5[0:MTS, :], AF.Exp,
                                 bias=nmx[0:MTS, 0:1])
            ssum = msb.tile([128, 1], F32, tag="ssum")
            nc.vector.tensor_reduce(out=ssum[0:MTS, :], in_=e5[0:MTS, :],
                                    axis=mybir.AxisListType.X, op=ALU.add)
            rs = msb.tile([128, 1], F32, tag="rs")
            nc.vector.reciprocal(rs[0:MTS, :], ssum[0:MTS, :])
            w5 = msb.tile([128, 5], F32, tag="w5")
            nc.vector.tensor_scalar(out=w5[0:MTS, :], in0=e5[0:MTS, :],
                                    scalar1=rs[0:MTS, 0:1], scalar2=None, op0=ALU.mult)
            # z_agg = sum_r w_r * z_r ; feat = [z_agg, z_0]
            wz = msb.tile([128, 5 * 6], F32, tag="wz")
            zview = zb[0:MTS, :].rearrange("p (r w) -> p r w", w=8)[:, :, 0:6]
            nc.vector.tensor_tensor(
                out=wz[0:MTS, :].rearrange("p (r f) -> p r f", f=6),
                in0=zview,
                in1=w5[0:MTS, :].rearrange("p r -> p r ()").to_broadcast([MTS, 5, 6]),
                op=ALU.mult)
            feat = msb.tile([128, 2 * F], F32, tag="feat")
            nc.vector.tensor_reduce(
                out=feat[0:MTS, 0:6],
                in_=wz[0:MTS, :].rearrange("p (r f) -> p f r", f=6),
                axis=mybir.AxisListType.X, op=ALU.add)
            nc.vector.tensor_copy(feat[0:MTS, 6:12], zb[0:MTS, 0:6])
            # MLP head (tiny fp32 matmuls; PSUM tags reused down the chain)
            pft = mps.tile([128, 128], F32, tag="mlpA", space="PSUM")
            nc.tensor.transpose(out=pft[0:2 * F, 0:128], in_=feat[:, :],
                                identity=ident[:])
            featT = msb.tile([2 * F, 128], F32, tag="featT")
            nc.scalar.activation(featT[:], pft[0:2 * F, 0:128], AF.Copy)
            ph1 = mps.tile([128, 128], F32, tag="mlpB", space="PSUM")
            nc.tensor.matmul(out=ph1[0:64, :], lhsT=w1t[:], rhs=featT[:],
                             start=True, stop=True)
            h1 = msb.tile([64, 128], F32, tag="h1")
            nc.scalar.activation(h1[:], ph1[0:64, :], AF.Relu, bias=b1c[:, 0:1])
            ph2 = mps.tile([128, 128], F32, tag="mlpA", space="PSUM")
            nc.tensor.matmul(out=ph2[0:32, :], lhsT=w2t[:], rhs=h1[:],
                             start=True, stop=True)
            h2 = msb.tile([32, 128], F32, tag="h2")
            nc.scalar.activation(h2[:], ph2[0:32, :], AF.Relu, bias=b2c[:, 0:1])
            py_ = mps.tile([128, 128], F32, tag="mlpB", space="PSUM")
            nc.tensor.matmul(out=py_[0:1, :], lhsT=w3t[:], rhs=h2[:],
                             start=True, stop=True)
            yrow = msb.tile([1, 128], F32, tag="yrow")
            nc.scalar.activation(yrow[:], py_[0:1, :], AF.Identity, bias=b3c[0:1, 0:1])
            pyt = mps.tile([128, 128], F32, tag="mlpA", space="PSUM")
            nc.tensor.transpose(out=pyt[:, 0:1], in_=yrow[0:1, :],
                                identity=ident[0:1, 0:1])
            ycol = msb.tile([128, 1], F32, tag="ycol")
            nc.vector.tensor_copy(ycol[:], pyt[:, 0:1])
            nc.gpsimd.indirect_dma_start(
                out=d_y[:, :],
                out_offset=IndirectOffsetOnAxis(ap=orows[0:MTS, g:g + 1], axis=0),
                in_=ycol[0:MTS, :], in_offset=None)

        for g in range(NG):
            for T8 in range(GT):
                T = GT * g + T8
                # per-tile transposed queries (gathers pipeline on Pool).
                # Group 0's targets reference only A-rows by construction;
                # later tiles mix A and B rows, filled by two bounds-checked
                # gathers (out-of-bounds offsets are skipped silently).
                qrow = gsb.tile([128, 128], F32, tag="qrow")
                if g == 0:
                    nc.gpsimd.indirect_dma_start(
                        out=qrow[:], out_offset=None, in_=d_qbaA[:, :],
                        in_offset=IndirectOffsetOnAxis(ap=trowsA[:, T:T + 1], axis=0))
                else:
                    nc.gpsimd.indirect_dma_start(
                        out=qrow[:], out_offset=None, in_=d_qbaA[:, :],
                        in_offset=IndirectOffsetOnAxis(ap=trowsA[:, T:T + 1], axis=0),
                        bounds_check=ND * aw - 1, oob_is_err=False)
                    nc.gpsimd.indirect_dma_start(
                        out=qrow[:], out_offset=None, in_=d_qbaB[:, :],
                        in_offset=IndirectOffsetOnAxis(ap=trowsB[:, T:T + 1], axis=0),
                        bounds_check=ND * (SS - aw + 1) - 1, oob_is_err=False)
                if g == 0 and T8 == GT - 1:
                    # all of group 0's gathers are issued: write the fence row
                    # (data-dependent on the last gather), then fire the late
                    # q exchange (completes well before group 1 needs it)
                    fence = gsb.tile([1, N], F32, tag="fence")
                    nc.gpsimd.tensor_copy(fence[:], qrow[0:1, :])
                    nc.sync.dma_start(out=d_qblB[bw:bw + 1, :], in_=fence[:])
                    nc.gpsimd.collective_compute(
                        "AllGather", ALU.bypass, replica_groups=groups,
                        ins=[d_qblB.ap().opt()], outs=[d_qbaB.ap().opt()])
                pt = gps_.tile([128, 128], F32, tag="pt", space="PSUM")
                nc.tensor.transpose(out=pt[:], in_=qrow[:], identity=ident[:])
                qT = qT_all[:, T * 128:(T + 1) * 128]
                nc.scalar.activation(qT, pt[:], AF.Copy)
                ssb = ssb_p.tile([128, COLS], F32, tag="ssb")
                for c0 in range(0, COLS, SCH):
                    c1 = min(COLS, c0 + SCH)
                    ps = sps.tile([128, SCH], F32, tag="sc", space="PSUM")
                    for b0 in range(0, c1 - c0, 512):
                        b1 = min(c1 - c0, b0 + 512)
                        nc.tensor.matmul(out=ps[:, b0:b1], lhsT=qT,
                                         rhs=keysT[:, c0 + b0:c0 + b1],
                                         start=True, stop=True)
                    nc.scalar.activation(ssb[:, c0:c1], ps[:, 0:c1 - c0], AF.Copy)
                nc.vector.max(out=v8[:, T * 8:(T + 1) * 8], in_=ssb[:])
                nc.vector.max_index(out=i8[:, T * 8:(T + 1) * 8],
                                    in_max=v8[:, T * 8:(T + 1) * 8], in_values=ssb[:])
            # merge previous group's exchanged candidates (its AllGather has
            # had a full group of score tiles to complete in)
            if g >= 1:
                merge_group(g - 1)
            # pack + exchange this group's candidates UNMASKED (the self
            # filter runs on the merge side, so the exchange depends only on
            # the scans and fires immediately)
            gsl = slice(g * GT * 8, (g + 1) * GT * 8)
            nc.sync.dma_start(
                out=cl_v[g][:, :, 0:8],
                in_=v8[:, gsl].rearrange("p (T w) -> p T w", w=8).bitcast(U32))
            nc.sync.dma_start(
                out=cl_v[g][:, :, 8:16],
                in_=i8[:, gsl].rearrange("p (T w) -> p T w", w=8))
            nc.gpsimd.collective_compute(
                "AllGather", ALU.bypass, replica_groups=groups,
                ins=[d_clg[g].ap().opt()], outs=[d_cag[g].ap().opt()])
        merge_group(NG - 1)

        mps.release()
        msb.release()
        sps.release()
        ssb_p.release()
        gps_.release()
        gsb.release()
        big.release()
        cpool.release()

    nc.compile()
    return nc


_CACHED_NC = {}
_BIGROW = 10_000_000


def _assignment(tix):
    """Per-owner target->(group, slot) assignment: each owner's 375 targets
    sorted by the referenced stock's local index lam = tix % SS; group g gets
    the g-th 125 of that order (so group 0 needs only low-lam q rows), and
    the A/B split threshold aw covers every group-0 lam."""
    asg = np.zeros((ND, NG, MTS), np.int64)
    aw = 126
    for d in range(ND):
        ts = np.arange(d * SS, (d + 1) * SS)
        order = np.argsort(tix[ts] % SS, kind="stable")
        for gg in range(NG):
            asg[d, gg] = ts[order[gg * MTS:(gg + 1) * MTS]]
        aw = max(aw, int((tix[asg[d, 0]] % SS).max()) + 1)
    assert aw <= 250, f"group-0 lam spread too wide: {aw}"
    return asg, aw


def _prep_inputs(X_scaled, X_raw, target_idx, lstm_Wih, lstm_Whh, lstm_bih,
                 lstm_bhh, ln_g, ln_b, WQ, WK, log_temp, lag_bias,
                 W1, b1, W2, b2, W3, b3):
    f32 = np.float32
    assert np.all(np.asarray(ln_b) == 0.0), "kernel assumes ln_b == 0"
    tix = np.asarray(target_idx).astype(np.int64)
    bias = (np.asarray(lstm_bih) + np.asarray(lstm_bhh)).astype(f32)
    gperm = np.r_[0:N, N:2 * N, 3 * N:4 * N, 2 * N:3 * N]    # [i, f, o, g]
    g_ln = np.asarray(ln_g).astype(f32)
    wq_f = (np.asarray(WQ) * g_ln[None, :]).astype(f32)
    wk_f = (np.asarray(WK) * g_ln[None, :]).astype(f32)
    uq = np.asarray(WQ) @ g_ln
    uk = np.asarray(WK) @ g_ln
    inv_temp = np.asarray(
        1.0 / np.clip(np.exp(np.asarray(log_temp, np.float64)), 0.1, np.sqrt(N)),
        f32).reshape(1, 1)

    # XZB table: flat (s,l) -> [6 raw feats at lag_pos, lag_bias, 0]
    Xr = np.asarray(X_raw)[0].astype(f32)                    # [S, L, F]
    lb = np.asarray(lag_bias).astype(f32)
    xzb = np.zeros((S * LMAX, 8), f32)
    lagpos = np.clip(L - 1 - (LMAX - np.arange(LMAX)), 0, L - 1)
    xzb[:, 0:6] = Xr[:, lagpos, :].reshape(S * LMAX, 6)
    xzb[:, 6] = np.tile(lb, S)

    # lam-sorted position assignment; target asg[d, g, o] sits at position
    # g*1024 + d*128 + o (tile T = 8g + d, row o)
    asg, aw = _assignment(tix)
    bw = SS - aw
    trowsA = np.full((128, NT), _BIGROW, np.uint32)
    trowsB = np.full((128, NT), _BIGROW, np.uint32)
    trowsA[:, 0:GT] = 0  # group-0 pad rows still gather a valid row
    gslo = np.full((128, NG, ND), -1e9, f32)
    orows = np.zeros((128, NG, ND), np.uint32)
    mrows = np.zeros((128, NG * ND, ND), np.uint32)
    for d in range(ND):
        for gg in range(NG):
            for o in range(MTS):
                t = asg[d, gg, o]
                sig = int(tix[t])
                ow, lam = divmod(sig, SS)
                T = GT * gg + d
                if lam < aw:
                    trowsA[o, T] = ow * aw + lam
                    trowsB[o, T] = _BIGROW
                else:
                    assert gg > 0
                    trowsA[o, T] = _BIGROW
                    trowsB[o, T] = ow * (bw + 1) + (lam - aw)
                gslo[o, gg, d] = sig * LMAX + 4.5
                orows[o, gg, d] = t
            for s in range(ND):
                mrows[:MTS, gg * ND + s, d] = s * GPOS + d * 128 + np.arange(MTS)

    Xs = np.asarray(X_scaled)[0].astype(f32)                 # [S, L, F]
    common = dict(
        wih_t=np.ascontiguousarray(np.vstack([
            np.asarray(lstm_Wih).astype(f32).T, bias[None, :]])[:, gperm]),
        whh_t=np.ascontiguousarray(np.asarray(lstm_Whh).astype(f32).T[:, gperm]),
        wq_t=np.ascontiguousarray(wq_f.T), wk_t=np.ascontiguousarray(wk_f.T),
        negu_q=np.ascontiguousarray((-uq.astype(f32) / N).reshape(1, N)),
        negu_k=np.ascontiguousarray((-uk.astype(f32) / N).reshape(1, N)),
        invt=inv_temp, xzb=xzb, tgtrowsA=trowsA, tgtrowsB=trowsB,
        w1_t=np.ascontiguousarray(np.asarray(W1).astype(f32).T),
        w2_t=np.ascontiguousarray(np.asarray(W2).astype(f32).T),
        w3_t=np.ascontiguousarray(np.asarray(W3).astype(f32).T),
        b1c=np.asarray(b1).astype(f32).reshape(64, 1),
        b2c=np.asarray(b2).astype(f32).reshape(32, 1),
        b3c=np.asarray(b3).astype(f32).reshape(1, 1),
    )
    in_maps = []
    for d in range(ND):
        # time-major xt: [F+1, L*SS], column = t*SS + s
        xtv = np.ascontiguousarray(np.vstack([
            Xs[d * SS:(d + 1) * SS].transpose(2, 1, 0).reshape(F, L * SS),
            np.ones((1, L * SS), f32)]))
        in_maps.append(dict(
            common, xt=xtv,
            gslo=np.ascontiguousarray(gslo[:, :, d]),
            mrows=np.ascontiguousarray(mrows[:, :, d]),
            orows=np.ascontiguousarray(orows[:, :, d]),
        ))
    return in_maps


def kernel(**inputs):
    tix = np.asarray(inputs["target_idx"]).astype(np.int64)
    _, aw = _assignment(tix)
    if aw not in _CACHED_NC:
        _CACHED_NC[aw] = build_program(aw)
    nc = _CACHED_NC[aw]
    in_maps = _prep_inputs(**inputs)
    res = run_bass_kernel_spmd(nc, in_maps, core_ids=list(range(ND)))
    y = np.zeros(S, np.float32)
    for d in range(ND):
        y[d * SS:(d + 1) * SS] = res.results[d]["y"][d * SS:(d + 1) * SS, 0]
    return y


# revision 62
# speedup vs baseline: 1.0369x; 1.0038x over previous
"""Trainium2 Bass kernel for nn_DeltaLag (LSTM encoder + lagged cross-attention
top-k + MLP head), distributed over 8 NeuronCores.

Sharding: stocks are split 375/core (LSTM + keys local to each core); every
core computes the score block [3072 padded target positions x 3750 local
(stock,lag) columns] in fp32, takes its local top-8 per target, and candidate
(value, index) pairs are exchanged with three pipelined AllToAlls (one per
8-tile position group) so the exchange and the per-group merge overlap the
next group's score computation. Each core merges + finishes its own 375
targets (z-gather + softmax + MLP).

Position layout: target t (owner d = t//375, i = t%375, g = i//125, o=i%125)
lives at position g*1024 + d*128 + o, i.e. tile T = 8g + d, row o. A group's
AllGather over rows [g*1024, (g+1)*1024) delivers core d's targets'
candidates from every source core at rows s*1024 + d*128 + o.

The compiled program is identical on all 8 cores (SPMD); everything
device-specific (shards, self-column ids, gather indices) is passed as input
tensors. All matmuls run in true fp32 (fp32r measured at ~1e-3 relative error
on this hardware, which would flip top-k selections).
"""

import sys

sys.path.insert(0, "/opt/trn_rl_repo")

import numpy as np

import concourse.bacc as bacc
import concourse.mybir as mybir
import concourse.tile as tile
from concourse.bass import IndirectOffsetOnAxis
from concourse.bass_utils import run_bass_kernel_spmd
from concourse.masks import make_identity

F32 = mybir.dt.float32
U32 = mybir.dt.uint32
U16 = mybir.dt.uint16
AF = mybir.ActivationFunctionType
ALU = mybir.AluOpType

S, F, N, L, LMAX, K = 3000, 6, 128, 40, 10, 5
ND = 8                      # cores
SS = S // ND                # stocks per core
COLS = SS * LMAX            # score columns per core
NG = 3                      # candidate-exchange groups
GPOS = ND * 128             # positions per group (1024)
NPOS = NG * GPOS            # padded target count (3072)
NT = NPOS // 128            # target tiles (24)
GT = NT // NG               # tiles per group (8)
MTS = 125                   # used rows per (group, owner) slot
SCH = 1024                  # score-tile PSUM chunk width
XCH = 8                     # xt DMA chunks (5 timesteps each)
CW16 = 16                   # u32 words per exchanged candidate row (8 v + 8 idx)


def build_program(aw):
    """aw: per-core stock-index threshold splitting the q AllGather into an
    early small exchange (rows [0,aw), enough for group-0's targets) and a
    late one hidden under group-0's score scans."""
    assert 125 <= aw <= 250
    bw = SS - aw
    nc = bacc.Bacc("TRN2", target_bir_lowering=False, debug=False,
                   enable_asserts=True, num_devices=ND)

    # ---- I/O ----
    d_xt = nc.dram_tensor("xt", [F + 1, L * SS], F32, kind="ExternalInput")
    d_wih = nc.dram_tensor("wih_t", [F + 1, 4 * N], F32, kind="ExternalInput")
    d_whh = nc.dram_tensor("whh_t", [N, 4 * N], F32, kind="ExternalInput")
    d_wqt = nc.dram_tensor("wq_t", [N, N], F32, kind="ExternalInput")
    d_wkt = nc.dram_tensor("wk_t", [N, N], F32, kind="ExternalInput")
    d_nuq = nc.dram_tensor("negu_q", [1, N], F32, kind="ExternalInput")
    d_nuk = nc.dram_tensor("negu_k", [1, N], F32, kind="ExternalInput")
    d_invt = nc.dram_tensor("invt", [1, 1], F32, kind="ExternalInput")
    d_gslo = nc.dram_tensor("gslo", [128, NG], F32, kind="ExternalInput")
    d_trowsA = nc.dram_tensor("tgtrowsA", [128, NT], U32, kind="ExternalInput")
    d_trowsB = nc.dram_tensor("tgtrowsB", [128, NT], U32, kind="ExternalInput")
    d_xzb = nc.dram_tensor("xzb", [S * LMAX, 8], F32, kind="ExternalInput")
    d_mrows = nc.dram_tensor("mrows", [128, NG * ND], U32, kind="ExternalInput")
    d_orows = nc.dram_tensor("orows", [128, NG], U32, kind="ExternalInput")
    d_w1t = nc.dram_tensor("w1_t", [2 * F, 64], F32, kind="ExternalInput")
    d_w2t = nc.dram_tensor("w2_t", [64, 32], F32, kind="ExternalInput")
    d_w3t = nc.dram_tensor("w3_t", [32, 1], F32, kind="ExternalInput")
    d_b1 = nc.dram_tensor("b1c", [64, 1], F32, kind="ExternalInput")
    d_b2 = nc.dram_tensor("b2c", [32, 1], F32, kind="ExternalInput")
    d_b3 = nc.dram_tensor("b3c", [1, 1], F32, kind="ExternalInput")

    d_y = nc.dram_tensor("y", [S, 1], F32, kind="ExternalOutput")

    # d_qblB carries one extra fence row (bw): written only after group 0's
    # q gathers are issued, so the B exchange cannot jump ahead of them on
    # the Pool engine's ready queue and delay group 0's scores.
    d_qblA = nc.dram_tensor("qb_localA", [aw, N], F32)
    d_qblB = nc.dram_tensor("qb_localB", [bw + 1, N], F32)
    d_qbaA = nc.dram_tensor("qb_allA", [ND * aw, N], F32, addr_space="Shared")
    d_qbaB = nc.dram_tensor("qb_allB", [ND * (bw + 1), N], F32,
                            addr_space="Shared")
    d_clg = [nc.dram_tensor(f"cand_local{g}", [GPOS, CW16], U32)
             for g in range(NG)]
    d_cag = [nc.dram_tensor(f"cand_all{g}", [ND * GPOS, CW16], U32,
                            addr_space="Shared") for g in range(NG)]

    groups = [list(range(ND))]

    with tile.TileContext(nc) as tc:
        cpool = tc.alloc_tile_pool(name="const", bufs=1)
        big = tc.alloc_tile_pool(name="big", bufs=1)

        # ---- constants / params to SBUF ----
        ident = cpool.tile([128, 128], F32)
        make_identity(nc, ident[:])
        ones1 = cpool.tile([1, 128], F32)
        nc.vector.memset(ones1[:], 1.0)
        onesf = cpool.tile([128, 128], F32)
        nc.vector.memset(onesf[:], 1.0)

        def load(pool, dram, shape, dtype=F32):
            t = pool.tile(shape, dtype, tag=f"ld_{dram.name}")
            nc.sync.dma_start(out=t[:], in_=dram[:, :])
            return t

        # LSTM weights and the ppre inputs first (the SP DMA queue is
        # in-order; PE's first scheduled ops are the ppre matmuls and the
        # first LSTM step, which need these plus only the first xt chunk)
        wih = load(cpool, d_wih, [F + 1, 4 * N])
        whh = load(cpool, d_whh, [N, 4 * N])
        nuq = load(cpool, d_nuq, [1, N])
        nuk = load(cpool, d_nuk, [1, N])
        # time-major xt arrives in XCH separate chunk tiles so the LSTM's
        # step-t matmul depends only on its own chunk's DMA
        TPC = L // XCH
        CW = TPC * SS
        xts = []
        for c in range(XCH):
            xtc = big.tile([F + 1, CW], F32, tag=f"xt{c}")
            nc.sync.dma_start(out=xtc[:], in_=d_xt[:, c * CW:(c + 1) * CW])
            xts.append(xtc)
        wqt = load(cpool, d_wqt, [N, N])
        wkt = load(cpool, d_wkt, [N, N])
        invt = load(cpool, d_invt, [1, 1])
        trowsA = load(cpool, d_trowsA, [128, NT], U32)
        trowsB = load(cpool, d_trowsB, [128, NT], U32)
        mrows = load(cpool, d_mrows, [128, NG * ND], U32)
        orows = load(cpool, d_orows, [128, NG], U32)
        w1t = load(cpool, d_w1t, [2 * F, 64])
        w2t = load(cpool, d_w2t, [64, 32])
        w3t = load(cpool, d_w3t, [32, 1])
        b1c = load(cpool, d_b1, [64, 1])
        b2c = load(cpool, d_b2, [32, 1])
        b3c = load(cpool, d_b3, [1, 1])

        gslo = load(cpool, d_gslo, [128, NG])
        invtb = cpool.tile([128, 1], F32)
        nc.gpsimd.partition_broadcast(invtb[:], invt[:], channels=128)

        iota_u = cpool.tile([128, 64], U32)
        nc.gpsimd.iota(iota_u[:], pattern=[[1, 64]], base=0, channel_multiplier=0)
        iota_f = cpool.tile([128, 64], F32)
        nc.vector.tensor_copy(iota_f[:], iota_u[:])
        base_u = cpool.tile([128, 64], U32)
        nc.gpsimd.iota(base_u[:], pattern=[[COLS, 8], [0, 8]], base=0,
                       channel_multiplier=0)
        base_f = cpool.tile([128, 64], F32)
        nc.vector.tensor_copy(base_f[:], base_u[:])

        # rank-1 LN-fold correction matrices: rows n, cols p -> -u[p]/128
        with tc.tile_pool(name="ppre", bufs=1, space="PSUM") as ppre:
            uqo = cpool.tile([128, 128], F32)
            uko = cpool.tile([128, 128], F32)
            pq = ppre.tile([128, 128], F32, space="PSUM")
            nc.tensor.matmul(out=pq[:], lhsT=ones1[:], rhs=nuq[:], start=True, stop=True)
            nc.scalar.activation(uqo[:], pq[:], AF.Copy)
            pk = ppre.tile([128, 128], F32, space="PSUM")
            nc.tensor.matmul(out=pk[:], lhsT=ones1[:], rhs=nuk[:], start=True, stop=True)
            nc.scalar.activation(uko[:], pk[:], AF.Copy)

        # ---- Phase 1: LSTM over the 375 local stocks ----
        # h,c layout [n=128, s]; last-10 hidden states land in hsave[n, s*10+k].
        # Gate columns in wih/whh are host-permuted to [i, f, o, g]; the bias
        # is folded into the xproj matmul via xt's constant-1 row.
        hsave = big.tile([128, COLS], F32)
        with tc.tile_pool(name="lstm_sb", bufs=2) as lsb, \
             tc.tile_pool(name="lstm_c", bufs=2) as lcp, \
             tc.tile_pool(name="lstm_ps", bufs=2, space="PSUM") as lps:
            h0 = lsb.tile([128, SS], F32, tag="h0")
            nc.vector.memset(h0[:], 0.0)
            c_prev = lcp.tile([128, SS], F32, tag="c")
            nc.vector.memset(c_prev[:], 0.0)
            h_prev = h0[:]
            for t in range(L):
                xs = xts[t // TPC][:, (t % TPC) * SS:(t % TPC + 1) * SS]
                # all four x-projections first: they don't depend on h_{t-1},
                # so the PE has work while the gate elementwise chain finishes
                gps = []
                for g in range(4):
                    psg = lps.tile([128, SS], F32, tag=f"g{g}", space="PSUM")
                    nc.tensor.matmul(out=psg[:], lhsT=wih[:, g * N:(g + 1) * N],
                                     rhs=xs, start=True, stop=False)
                    gps.append(psg)
                for g in range(4):
                    nc.tensor.matmul(out=gps[g][:], lhsT=whh[:, g * N:(g + 1) * N],
                                     rhs=h_prev, start=False, stop=True)
                si = lsb.tile([128, SS], F32, tag="si")
                nc.scalar.activation(si[:], gps[0][:], AF.Sigmoid)
                sf = lsb.tile([128, SS], F32, tag="sf")
                nc.scalar.activation(sf[:], gps[1][:], AF.Sigmoid)
                so_t = lsb.tile([128, SS], F32, tag="so")
                nc.scalar.activation(so_t[:], gps[2][:], AF.Sigmoid)
                tg = lsb.tile([128, SS], F32, tag="tg")
                nc.scalar.activation(tg[:], gps[3][:], AF.Tanh)
                si, sf, so = si[:], sf[:], so_t[:]
                t1 = lsb.tile([128, SS], F32, tag="t1")
                nc.vector.tensor_tensor(out=t1[:], in0=si, in1=tg[:], op=ALU.mult)
                c2 = lsb.tile([128, SS], F32, tag="c2")
                nc.gpsimd.tensor_tensor(out=c2[:], in0=sf, in1=c_prev[:], op=ALU.mult)
                c_new = lcp.tile([128, SS], F32, tag="c")
                nc.vector.tensor_tensor(out=c_new[:], in0=c2[:], in1=t1[:], op=ALU.add)
                th = lsb.tile([128, SS], F32, tag="th")
                nc.scalar.activation(th[:], c_new[:], AF.Tanh)
                if t >= L - LMAX:
                    h_out = hsave[:, (t - (L - LMAX))::LMAX]
                    nc.gpsimd.tensor_tensor(out=h_out, in0=so, in1=th[:], op=ALU.mult)
                    h_prev = h_out
                else:
                    hn = lsb.tile([128, SS], F32, tag="hn")
                    nc.gpsimd.tensor_tensor(out=hn[:], in0=so, in1=th[:], op=ALU.mult)
                    h_prev = hn[:]
                c_prev = c_new

        # ---- Phase 2: queries first, so the AllGather overlaps the keys ----
        with tc.tile_pool(name="q_sb", bufs=2) as qsb, \
             tc.tile_pool(name="q_ps", bufs=2, space="PSUM") as qps:
            h39 = hsave[:, (LMAX - 1)::LMAX]
            pyq = qps.tile([128, SS], F32, tag="yq", space="PSUM")
            nc.tensor.matmul(out=pyq[:], lhsT=wqt[:], rhs=h39, start=True, stop=False)
            nc.tensor.matmul(out=pyq[:], lhsT=uqo[:], rhs=h39, start=False, stop=True)
            yq = qsb.tile([128, SS], F32, tag="yq_sb")
            nc.scalar.activation(yq[:], pyq[:], AF.Copy)
            y2q = qsb.tile([128, SS], F32, tag="y2q")
            nc.scalar.activation(y2q[:], pyq[:], AF.Square)
            psq = qps.tile([128, SS], F32, tag="sq", space="PSUM")
            nc.tensor.matmul(out=psq[:], lhsT=onesf[:], rhs=y2q[:], start=True, stop=True)
            sq = qsb.tile([128, SS], F32, tag="sqq")
            nc.scalar.sqrt(sq[:], psq[:])
            ri = qsb.tile([128, SS], F32, tag="riq")
            nc.vector.reciprocal(ri[:], sq[:])
            qn1 = qsb.tile([128, SS], F32, tag="qn1")
            nc.vector.tensor_tensor(out=qn1[:], in0=yq[:], in1=ri[:], op=ALU.mult)
            qn = qsb.tile([128, SS], F32, tag="qn")
            nc.vector.tensor_scalar(out=qn[:], in0=qn1[:], scalar1=invtb[:, 0:1],
                                    scalar2=None, op0=ALU.mult)
            for j in range(3):
                pt = qps.tile([128, 128], F32, tag="qt", space="PSUM")
                nc.tensor.transpose(out=pt[0:MTS, :], in_=qn[:, j * MTS:(j + 1) * MTS],
                                    identity=ident[:])
                qrow_sb = qsb.tile([128, 128], F32, tag="qrow_sb")
                nc.scalar.activation(qrow_sb[0:MTS, :], pt[0:MTS, :], AF.Copy)
                # split the q rows at local stock index `aw` between the
                # early (A) and late (B) exchange tensors
                lo, hi = j * MTS, j * MTS + MTS
                if hi <= aw:
                    nc.sync.dma_start(out=d_qblA[lo:hi, :], in_=qrow_sb[0:MTS, :])
                elif lo >= aw:
                    nc.sync.dma_start(out=d_qblB[lo - aw:hi - aw, :],
                                      in_=qrow_sb[0:MTS, :])
                else:
                    na = aw - lo
                    nc.sync.dma_start(out=d_qblA[lo:aw, :], in_=qrow_sb[0:na, :])
                    nc.sync.dma_start(out=d_qblB[0:hi - aw, :],
                                      in_=qrow_sb[na:MTS, :])
                if hi >= aw and lo < aw:
                    # all A rows written: fire the early exchange now
                    nc.gpsimd.collective_compute(
                        "AllGather", ALU.bypass, replica_groups=groups,
                        ins=[d_qblA.ap().opt()], outs=[d_qbaA.ap().opt()])

        # ---- Phase 3: keys (LN+l2norm folded into matmuls; sigma cancels) ----
        keysT = big.tile([128, COLS], F32)
        with tc.tile_pool(name="key_sb", bufs=3) as ksb, \
             tc.tile_pool(name="key_ps", bufs=2, space="PSUM") as kps:
            ysb = big.tile([128, COLS], F32)
            CH = 512
            chunks = [(c0, min(COLS, c0 + CH)) for c0 in range(0, COLS, CH)]
            sqs = []
            for c0, c1 in chunks:
                w = c1 - c0
                py = kps.tile([128, CH], F32, tag="y", space="PSUM")
                nc.tensor.matmul(out=py[:, :w], lhsT=wkt[:],
                                 rhs=hsave[:, c0:c1], start=True, stop=False)
                nc.tensor.matmul(out=py[:, :w], lhsT=uko[:],
                                 rhs=hsave[:, c0:c1], start=False, stop=True)
                nc.scalar.activation(ysb[:, c0:c1], py[:, :w], AF.Copy)
                y2 = ksb.tile([128, CH], F32, tag="y2")
                nc.scalar.activation(y2[:, :w], py[:, :w], AF.Square)
                psq = kps.tile([128, CH], F32, tag="s", space="PSUM")
                nc.tensor.matmul(out=psq[:, :w], lhsT=onesf[:],
                                 rhs=y2[:, :w], start=True, stop=True)
                sq = ksb.tile([128, CH], F32, tag="sq")
                nc.scalar.sqrt(sq[:, :w], psq[:, :w])
                sqs.append((sq, c0, c1))
            for sq, c0, c1 in sqs:
                w = c1 - c0
                ri = ksb.tile([128, CH], F32, tag="ri")
                nc.vector.reciprocal(ri[:, :w], sq[:, :w])
                nc.vector.tensor_tensor(out=keysT[:, c0:c1], in0=ysb[:, c0:c1],
                                        in1=ri[:, :w], op=ALU.mult)

        # ---- Phase 4+5+6: per group: qT gather/transpose + scores + top-8
        # (per tile), candidate AllGather exchange, and the merge of the
        # PREVIOUS group's exchanged candidates ----
        qT_all = big.tile([128, NPOS], F32)
        v8 = big.tile([128, NT * 8], F32)
        i8 = big.tile([128, NT * 8], U32)
        cl_v = [d_clg[g].ap().rearrange("(T p) w -> p T w", p=128)
                for g in range(NG)]

        qrow0 = big.tile([128, GT * 128], F32)
        gsb = tc.alloc_tile_pool(name="qg_sb", bufs=4)
        gps_ = tc.alloc_tile_pool(name="qg_ps", bufs=2, space="PSUM")
        ssb_p = tc.alloc_tile_pool(name="sc_sb", bufs=2)
        sps = tc.alloc_tile_pool(name="sc_ps", bufs=2, space="PSUM")
        msb = tc.alloc_tile_pool(name="m_sb", bufs=2)
        mps = tc.alloc_tile_pool(name="m_ps", bufs=1, space="PSUM")

        def merge_group(g):
            # candidates for my 125 targets of group g, from all 8 cores
            mv = msb.tile([128, ND * CW16], U32, tag="mv")
            for s in range(ND):
                nc.gpsimd.indirect_dma_start(
                    out=mv[0:MTS, s * CW16:(s + 1) * CW16], out_offset=None,
                    in_=d_cag[g][:, :],
                    in_offset=IndirectOffsetOnAxis(
                        ap=mrows[0:MTS, g * ND + s:g * ND + s + 1], axis=0))
            mvals = mv[0:MTS, :].bitcast(F32).rearrange(
                "p (d w) -> p d w", w=CW16)[:, :, 0:8]
            midx = mv[0:MTS, :].rearrange("p (d w) -> p d w", w=CW16)[:, :, 8:16]
            # leading copies + self-filter arithmetic on gpsimd so the DVE's
            # scan stream is disturbed as little as possible
            mvalc = msb.tile([128, 64], F32, tag="mvalc")
            nc.gpsimd.tensor_copy(mvalc[0:MTS, :], mvals)
            # global flat candidate index = owner*COLS + local
            midxf = msb.tile([128, 64], F32, tag="midxf")
            nc.gpsimd.tensor_copy(midxf[0:MTS, :], midx)
            nc.gpsimd.tensor_tensor(out=midxf[0:MTS, :], in0=midxf[0:MTS, :],
                                    in1=base_f[0:MTS, :], op=ALU.add)
            # self-stock filter (candidates were exchanged unmasked):
            # self iff |gidx - (10*self_stock + 4.5)| < 5
            dm = msb.tile([128, 64], F32, tag="dm")
            nc.gpsimd.tensor_tensor(
                out=dm[0:MTS, :], in0=midxf[0:MTS, :],
                in1=gslo[0:MTS, g:g + 1].to_broadcast([MTS, 64]),
                op=ALU.subtract)
            nc.gpsimd.tensor_tensor(out=dm[0:MTS, :], in0=dm[0:MTS, :],
                                    in1=dm[0:MTS, :], op=ALU.mult)
            nc.vector.tensor_scalar(out=dm[0:MTS, :], in0=dm[0:MTS, :],
                                    scalar1=25.0, scalar2=None, op0=ALU.is_lt)
            nc.vector.scalar_tensor_tensor(out=mvalc[0:MTS, :], in0=dm[0:MTS, :],
                                           scalar=-1e30, in1=mvalc[0:MTS, :],
                                           op0=ALU.mult, op1=ALU.add)
            v8m = msb.tile([128, 8], F32, tag="v8m")
            nc.vector.max(out=v8m[0:MTS, :], in_=mvalc[0:MTS, :])
            pos8 = msb.tile([128, 8], U32, tag="pos8")
            nc.vector.max_index(out=pos8[0:MTS, :], in_max=v8m[0:MTS, :],
                                in_values=mvalc[0:MTS, :])
            pos5f = msb.tile([128, 5], F32, tag="pos5f")
            nc.vector.tensor_copy(pos5f[0:MTS, :], pos8[0:MTS, 0:5])
            eq = msb.tile([128, 5 * 64], F32, tag="eq")
            eq_v = eq[0:MTS, :].rearrange("p (k c) -> p k c", c=64)
            nc.vector.tensor_tensor(
                out=eq_v,
                in0=pos5f[0:MTS, :].rearrange("p k -> p k ()").to_broadcast([MTS, 5, 64]),
                in1=iota_f[0:MTS, :].rearrange("p c -> p () c").to_broadcast([MTS, 5, 64]),
                op=ALU.is_equal)
            nc.vector.tensor_tensor(
                out=eq_v, in0=eq_v,
                in1=midxf[0:MTS, :].rearrange("p c -> p () c").to_broadcast([MTS, 5, 64]),
                op=ALU.mult)
            g5f = msb.tile([128, 5], F32, tag="g5f")
            nc.vector.tensor_reduce(out=g5f[0:MTS, :], in_=eq_v,
                                    axis=mybir.AxisListType.X, op=ALU.add)
            g5u = msb.tile([128, 5], U32, tag="g5u")
            nc.vector.tensor_copy(g5u[0:MTS, :], g5f[0:MTS, :])
            # z rows (6 raw feats + lag_bias + pad) for the 5 winners
            zb = msb.tile([128, 5 * 8], F32, tag="zb")
            for r in range(K):
                nc.gpsimd.indirect_dma_start(
                    out=zb[0:MTS, r * 8:(r + 1) * 8], out_offset=None,
                    in_=d_xzb[:, :],
                    in_offset=IndirectOffsetOnAxis(ap=g5u[0:MTS, r:r + 1], axis=0))
            vb5 = msb.tile([128, 5], F32, tag="vb5")
            nc.vector.tensor_tensor(out=vb5[0:MTS, :], in0=v8m[0:MTS, 0:5],
                                    in1=zb[0:MTS, 6::8], op=ALU.add)
            # softmax over the 5 candidate scores
            mx = msb.tile([128, 1], F32, tag="mx")
            nc.vector.tensor_reduce(out=mx[0:MTS, :], in_=vb5[0:MTS, :],
                                    axis=mybir.AxisListType.X, op=ALU.max)
            nmx = msb.tile([128, 1], F32, tag="nmx")
            nc.vector.tensor_scalar(out=nmx[0:MTS, :], in0=mx[0:MTS, :],
                                    scalar1=-1.0, scalar2=None, op0=ALU.mult)
            e5 = msb.tile([128, 5], F32, tag="e5")
            nc.scalar.activation(e5[0:MTS, :], vb5[0:MTS, :], AF.Exp,
                                 bias=nmx[0:MTS, 0:1])
            ssum = msb.tile([128, 1], F32, tag="ssum")
            nc.vector.tensor_reduce(out=ssum[0:MTS, :], in_=e5[0:MTS, :],
                                    axis=mybir.AxisListType.X, op=ALU.add)
            rs = msb.tile([128, 1], F32, tag="rs")
            nc.vector.reciprocal(rs[0:MTS, :], ssum[0:MTS, :])
            w5 = msb.tile([128, 5], F32, tag="w5")
            nc.vector.tensor_scalar(out=w5[0:MTS, :], in0=e5[0:MTS, :],
                                    scalar1=rs[0:MTS, 0:1], scalar2=None, op0=ALU.mult)
            # z_agg = sum_r w_r * z_r ; feat = [z_agg, z_0]
            wz = msb.tile([128, 5 * 6], F32, tag="wz")
            zview = zb[0:MTS, :].rearrange("p (r w) -> p r w", w=8)[:, :, 0:6]
            nc.vector.tensor_tensor(
                out=wz[0:MTS, :].rearrange("p (r f) -> p r f", f=6),
                in0=zview,
                in1=w5[0:MTS, :].rearrange("p r -> p r ()").to_broadcast([MTS, 5, 6]),
                op=ALU.mult)
            feat = msb.tile([128, 2 * F], F32, tag="feat")
            nc.vector.tensor_reduce(
                out=feat[0:MTS, 0:6],
                in_=wz[0:MTS, :].rearrange("p (r f) -> p f r", f=6),
                axis=mybir.AxisListType.X, op=ALU.add)
            nc.vector.tensor_copy(feat[0:MTS, 6:12], zb[0:MTS, 0:6])
            # MLP head (tiny fp32 matmuls; PSUM tags reused down the chain)
            pft = mps.tile([128, 128], F32, tag="mlpA", space="PSUM")
            nc.tensor.transpose(out=pft[0:2 * F, 0:128], in_=feat[:, :],
                                identity=ident[:])
            featT = msb.tile([2 * F, 128], F32, tag="featT")
            nc.scalar.activation(featT[:], pft[0:2 * F, 0:128], AF.Copy)
            ph1 = mps.tile([128, 128], F32, tag="mlpB", space="PSUM")
            nc.tensor.matmul(out=ph1[0:64, :], lhsT=w1t[:], rhs=featT[:],
                             start=True, stop=True)
            h1 = msb.tile([64, 128], F32, tag="h1")
            nc.scalar.activation(h1[:], ph1[0:64, :], AF.Relu, bias=b1c[:, 0:1])
            ph2 = mps.tile([128, 128], F32, tag="mlpA", space="PSUM")
            nc.tensor.matmul(out=ph2[0:32, :], lhsT=w2t[:], rhs=h1[:],
                             start=True, stop=True)
            h2 = msb.tile([32, 128], F32, tag="h2")
            nc.scalar.activation(h2[:], ph2[0:32, :], AF.Relu, bias=b2c[:, 0:1])
            py_ = mps.tile([128, 128], F32, tag="mlpB", space="PSUM")
            nc.tensor.matmul(out=py_[0:1, :], lhsT=w3t[:], rhs=h2[:],
                             start=True, stop=True)
            yrow = msb.tile([1, 128], F32, tag="yrow")
            nc.scalar.activation(yrow[:], py_[0:1, :], AF.Identity, bias=b3c[0:1, 0:1])
            pyt = mps.tile([128, 128], F32, tag="mlpA", space="PSUM")
            nc.tensor.transpose(out=pyt[:, 0:1], in_=yrow[0:1, :],
                                identity=ident[0:1, 0:1])
            ycol = msb.tile([128, 1], F32, tag="ycol")
            nc.vector.tensor_copy(ycol[:], pyt[:, 0:1])
            nc.gpsimd.indirect_dma_start(
                out=d_y[:, :],
                out_offset=IndirectOffsetOnAxis(ap=orows[0:MTS, g:g + 1], axis=0),
                in_=ycol[0:MTS, :], in_offset=None)

        for g in range(NG):
            for T8 in range(GT):
                T = GT * g + T8
                # per-tile transposed queries (gathers pipeline on Pool).
                # Group 0's targets reference only A-rows by construction;
                # later tiles mix A and B rows, filled by two bounds-checked
                # gathers (out-of-bounds offsets are skipped silently).
                if g == 0:
                    # group 0 gathers land in one dedicated wide tile: no
                    # buffer recycling against the PE transposes, so all 8
                    # gathers and the fence run back-to-back right after the
                    # early q exchange
                    qrow = qrow0[:, T8 * 128:(T8 + 1) * 128]
                    nc.gpsimd.indirect_dma_start(
                        out=qrow, out_offset=None, in_=d_qbaA[:, :],
                        in_offset=IndirectOffsetOnAxis(ap=trowsA[:, T:T + 1], axis=0))
                else:
                    qrowt = gsb.tile([128, 128], F32, tag="qrow")
                    qrow = qrowt[:]
                    nc.gpsimd.indirect_dma_start(
                        out=qrow, out_offset=None, in_=d_qbaA[:, :],
                        in_offset=IndirectOffsetOnAxis(ap=trowsA[:, T:T + 1], axis=0),
                        bounds_check=ND * aw - 1, oob_is_err=False)
                    nc.gpsimd.indirect_dma_start(
                        out=qrow, out_offset=None, in_=d_qbaB[:, :],
                        in_offset=IndirectOffsetOnAxis(ap=trowsB[:, T:T + 1], axis=0),
                        bounds_check=ND * (SS - aw + 1) - 1, oob_is_err=False)
                if g == 0 and T8 == GT - 1:
                    # all of group 0's gathers are issued: write the fence row
                    # (data-dependent on the last gather), then fire the late
                    # q exchange (completes well before group 1 needs it)
                    fence = gsb.tile([1, N], F32, tag="fence")
                    nc.gpsimd.tensor_copy(
                        fence[:], qrow0[0:1, (GT - 1) * 128:GT * 128])
                    nc.sync.dma_start(out=d_qblB[bw:bw + 1, :], in_=fence[:])
                    nc.gpsimd.collective_compute(
                        "AllGather", ALU.bypass, replica_groups=groups,
                        ins=[d_qblB.ap().opt()], outs=[d_qbaB.ap().opt()])
                pt = gps_.tile([128, 128], F32, tag="pt", space="PSUM")
                nc.tensor.transpose(out=pt[:], in_=qrow, identity=ident[:])
                qT = qT_all[:, T * 128:(T + 1) * 128]
                nc.scalar.activation(qT, pt[:], AF.Copy)
                ssb = ssb_p.tile([128, COLS], F32, tag="ssb")
                for c0 in range(0, COLS, SCH):
                    c1 = min(COLS, c0 + SCH)
                    ps = sps.tile([128, SCH], F32, tag="sc", space="PSUM")
                    for b0 in range(0, c1 - c0, 512):
                        b1 = min(c1 - c0, b0 + 512)
                        nc.tensor.matmul(out=ps[:, b0:b1], lhsT=qT,
                                         rhs=keysT[:, c0 + b0:c0 + b1],
                                         start=True, stop=True)
                    nc.scalar.activation(ssb[:, c0:c1], ps[:, 0:c1 - c0], AF.Copy)
                nc.vector.max(out=v8[:, T * 8:(T + 1) * 8], in_=ssb[:])
                nc.vector.max_index(out=i8[:, T * 8:(T + 1) * 8],
                                    in_max=v8[:, T * 8:(T + 1) * 8], in_values=ssb[:])
            # merge previous group's exchanged candidates (its AllGather has
            # had a full group of score tiles to complete in)
            if g >= 1:
                merge_group(g - 1)
            # pack + exchange this group's candidates UNMASKED (the self
            # filter runs on the merge side, so the exchange depends only on
            # the scans and fires immediately)
            gsl = slice(g * GT * 8, (g + 1) * GT * 8)
            nc.sync.dma_start(
                out=cl_v[g][:, :, 0:8],
                in_=v8[:, gsl].rearrange("p (T w) -> p T w", w=8).bitcast(U32))
            nc.sync.dma_start(
                out=cl_v[g][:, :, 8:16],
                in_=i8[:, gsl].rearrange("p (T w) -> p T w", w=8))
            nc.gpsimd.collective_compute(
                "AllGather", ALU.bypass, replica_groups=groups,
                ins=[d_clg[g].ap().opt()], outs=[d_cag[g].ap().opt()])
        merge_group(NG - 1)

        mps.release()
        msb.release()
        sps.release()
        ssb_p.release()
        gps_.release()
        gsb.release()
        big.release()
        cpool.release()

    nc.compile()
    return nc


_CACHED_NC = {}
_BIGROW = 10_000_000


def _assignment(tix):
    """Per-owner target->(group, slot) assignment: each owner's 375 targets
    sorted by the referenced stock's local index lam = tix % SS; group g gets
    the g-th 125 of that order (so group 0 needs only low-lam q rows), and
    the A/B split threshold aw covers every group-0 lam."""
    asg = np.zeros((ND, NG, MTS), np.int64)
    aw = 126
    for d in range(ND):
        ts = np.arange(d * SS, (d + 1) * SS)
        order = np.argsort(tix[ts] % SS, kind="stable")
        for gg in range(NG):
            asg[d, gg] = ts[order[gg * MTS:(gg + 1) * MTS]]
        aw = max(aw, int((tix[asg[d, 0]] % SS).max()) + 1)
    assert aw <= 250, f"group-0 lam spread too wide: {aw}"
    return asg, aw


def _prep_inputs(X_scaled, X_raw, target_idx, lstm_Wih, lstm_Whh, lstm_bih,
                 lstm_bhh, ln_g, ln_b, WQ, WK, log_temp, lag_bias,
                 W1, b1, W2, b2, W3, b3):
    f32 = np.float32
    assert np.all(np.asarray(ln_b) == 0.0), "kernel assumes ln_b == 0"
    tix = np.asarray(target_idx).astype(np.int64)
    bias = (np.asarray(lstm_bih) + np.asarray(lstm_bhh)).astype(f32)
    gperm = np.r_[0:N, N:2 * N, 3 * N:4 * N, 2 * N:3 * N]    # [i, f, o, g]
    g_ln = np.asarray(ln_g).astype(f32)
    wq_f = (np.asarray(WQ) * g_ln[None, :]).astype(f32)
    wk_f = (np.asarray(WK) * g_ln[None, :]).astype(f32)
    uq = np.asarray(WQ) @ g_ln
    uk = np.asarray(WK) @ g_ln
    inv_temp = np.asarray(
        1.0 / np.clip(np.exp(np.asarray(log_temp, np.float64)), 0.1, np.sqrt(N)),
        f32).reshape(1, 1)

    # XZB table: flat (s,l) -> [6 raw feats at lag_pos, lag_bias, 0]
    Xr = np.asarray(X_raw)[0].astype(f32)                    # [S, L, F]
    lb = np.asarray(lag_bias).astype(f32)
    xzb = np.zeros((S * LMAX, 8), f32)
    lagpos = np.clip(L - 1 - (LMAX - np.arange(LMAX)), 0, L - 1)
    xzb[:, 0:6] = Xr[:, lagpos, :].reshape(S * LMAX, 6)
    xzb[:, 6] = np.tile(lb, S)

    # lam-sorted position assignment; target asg[d, g, o] sits at position
    # g*1024 + d*128 + o (tile T = 8g + d, row o)
    asg, aw = _assignment(tix)
    bw = SS - aw
    trowsA = np.full((128, NT), _BIGROW, np.uint32)
    trowsB = np.full((128, NT), _BIGROW, np.uint32)
    trowsA[:, 0:GT] = 0  # group-0 pad rows still gather a valid row
    gslo = np.full((128, NG, ND), -1e9, f32)
    orows = np.zeros((128, NG, ND), np.uint32)
    mrows = np.zeros((128, NG * ND, ND), np.uint32)
    for d in range(ND):
        for gg in range(NG):
            for o in range(MTS):
                t = asg[d, gg, o]
                sig = int(tix[t])
                ow, lam = divmod(sig, SS)
                T = GT * gg + d
                if lam < aw:
                    trowsA[o, T] = ow * aw + lam
                    trowsB[o, T] = _BIGROW
                else:
                    assert gg > 0
                    trowsA[o, T] = _BIGROW
                    trowsB[o, T] = ow * (bw + 1) + (lam - aw)
                gslo[o, gg, d] = sig * LMAX + 4.5
                orows[o, gg, d] = t
            for s in range(ND):
                mrows[:MTS, gg * ND + s, d] = s * GPOS + d * 128 + np.arange(MTS)

    Xs = np.asarray(X_scaled)[0].astype(f32)                 # [S, L, F]
    common = dict(
        wih_t=np.ascontiguousarray(np.vstack([
            np.asarray(lstm_Wih).astype(f32).T, bias[None, :]])[:, gperm]),
        whh_t=np.ascontiguousarray(np.asarray(lstm_Whh).astype(f32).T[:, gperm]),
        wq_t=np.ascontiguousarray(wq_f.T), wk_t=np.ascontiguousarray(wk_f.T),
        negu_q=np.ascontiguousarray((-uq.astype(f32) / N).reshape(1, N)),
        negu_k=np.ascontiguousarray((-uk.astype(f32) / N).reshape(1, N)),
        invt=inv_temp, xzb=xzb, tgtrowsA=trowsA, tgtrowsB=trowsB,
        w1_t=np.ascontiguousarray(np.asarray(W1).astype(f32).T),
        w2_t=np.ascontiguousarray(np.asarray(W2).astype(f32).T),
        w3_t=np.ascontiguousarray(np.asarray(W3).astype(f32).T),
        b1c=np.asarray(b1).astype(f32).reshape(64, 1),
        b2c=np.asarray(b2).astype(f32).reshape(32, 1),
        b3c=np.asarray(b3).astype(f32).reshape(1, 1),
    )
    in_maps = []
    for d in range(ND):
        # time-major xt: [F+1, L*SS], column = t*SS + s
        xtv = np.ascontiguousarray(np.vstack([
            Xs[d * SS:(d + 1) * SS].transpose(2, 1, 0).reshape(F, L * SS),
            np.ones((1, L * SS), f32)]))
        in_maps.append(dict(
            common, xt=xtv,
            gslo=np.ascontiguousarray(gslo[:, :, d]),
            mrows=np.ascontiguousarray(mrows[:, :, d]),
            orows=np.ascontiguousarray(orows[:, :, d]),
        ))
    return in_maps


def kernel(**inputs):
    tix = np.asarray(inputs["target_idx"]).astype(np.int64)
    _, aw = _assignment(tix)
    if aw not in _CACHED_NC:
        _CACHED_NC[aw] = build_program(aw)
    nc = _CACHED_NC[aw]
    in_maps = _prep_inputs(**inputs)
    res = run_bass_kernel_spmd(nc, in_maps, core_ids=list(range(ND)))
    y = np.zeros(S, np.float32)
    for d in range(ND):
        y[d * SS:(d + 1) * SS] = res.results[d]["y"][d * SS:(d + 1) * SS, 0]
    return y


# revision 65
# speedup vs baseline: 1.0389x; 1.0020x over previous
"""Trainium2 Bass kernel for nn_DeltaLag (LSTM encoder + lagged cross-attention
top-k + MLP head), distributed over 8 NeuronCores.

Sharding: stocks are split 375/core (LSTM + keys local to each core); every
core computes the score block [3072 padded target positions x 3750 local
(stock,lag) columns] in fp32, takes its local top-8 per target, and candidate
(value, index) pairs are exchanged with three pipelined AllToAlls (one per
8-tile position group) so the exchange and the per-group merge overlap the
next group's score computation. Each core merges + finishes its own 375
targets (z-gather + softmax + MLP).

Position layout: target t (owner d = t//375, i = t%375, g = i//125, o=i%125)
lives at position g*1024 + d*128 + o, i.e. tile T = 8g + d, row o. A group's
AllGather over rows [g*1024, (g+1)*1024) delivers core d's targets'
candidates from every source core at rows s*1024 + d*128 + o.

The compiled program is identical on all 8 cores (SPMD); everything
device-specific (shards, self-column ids, gather indices) is passed as input
tensors. All matmuls run in true fp32 (fp32r measured at ~1e-3 relative error
on this hardware, which would flip top-k selections).
"""

import sys

sys.path.insert(0, "/opt/trn_rl_repo")

import numpy as np

import concourse.bacc as bacc
import concourse.mybir as mybir
import concourse.tile as tile
from concourse.bass import IndirectOffsetOnAxis
from concourse.bass_utils import run_bass_kernel_spmd
from concourse.masks import make_identity

F32 = mybir.dt.float32
U32 = mybir.dt.uint32
U16 = mybir.dt.uint16
AF = mybir.ActivationFunctionType
ALU = mybir.AluOpType

S, F, N, L, LMAX, K = 3000, 6, 128, 40, 10, 5
ND = 8                      # cores
SS = S // ND                # stocks per core
COLS = SS * LMAX            # score columns per core
NG = 3                      # candidate-exchange groups
GPOS = ND * 128             # positions per group (1024)
NPOS = NG * GPOS            # padded target count (3072)
NT = NPOS // 128            # target tiles (24)
GT = NT // NG               # tiles per group (8)
MTS = 125                   # used rows per (group, owner) slot
SCH = 1024                  # score-tile PSUM chunk width
XCH = 8                     # xt DMA chunks (5 timesteps each)
CW16 = 16                   # u32 words per exchanged candidate row (8 v + 8 idx)


def build_program(aw):
    """aw: per-core stock-index threshold splitting the q AllGather into an
    early small exchange (rows [0,aw), enough for group-0's targets) and a
    late one hidden under group-0's score scans."""
    assert 125 <= aw <= 250
    bw = SS - aw
    nc = bacc.Bacc("TRN2", target_bir_lowering=False, debug=False,
                   enable_asserts=True, num_devices=ND)

    # ---- I/O ----
    d_xt = nc.dram_tensor("xt", [F + 1, L * SS], F32, kind="ExternalInput")
    d_wih = nc.dram_tensor("wih_t", [F + 1, 4 * N], F32, kind="ExternalInput")
    d_whh = nc.dram_tensor("whh_t", [N, 4 * N], F32, kind="ExternalInput")
    d_wqt = nc.dram_tensor("wq_t", [N, N], F32, kind="ExternalInput")
    d_wkt = nc.dram_tensor("wk_t", [N, N], F32, kind="ExternalInput")
    d_nuq = nc.dram_tensor("negu_q", [1, N], F32, kind="ExternalInput")
    d_nuk = nc.dram_tensor("negu_k", [1, N], F32, kind="ExternalInput")
    d_invt = nc.dram_tensor("invt", [1, 1], F32, kind="ExternalInput")
    d_gslo = nc.dram_tensor("gslo", [128, NG], F32, kind="ExternalInput")
    d_trowsA = nc.dram_tensor("tgtrowsA", [128, NT], U32, kind="ExternalInput")
    d_trowsB = nc.dram_tensor("tgtrowsB", [128, NT], U32, kind="ExternalInput")
    d_xzb = nc.dram_tensor("xzb", [S * LMAX, 8], F32, kind="ExternalInput")
    d_mrows = nc.dram_tensor("mrows", [128, NG * ND], U32, kind="ExternalInput")
    d_orows = nc.dram_tensor("orows", [128, NG], U32, kind="ExternalInput")
    d_w1t = nc.dram_tensor("w1_t", [2 * F, 64], F32, kind="ExternalInput")
    d_w2t = nc.dram_tensor("w2_t", [64, 32], F32, kind="ExternalInput")
    d_w3t = nc.dram_tensor("w3_t", [32, 1], F32, kind="ExternalInput")
    d_b1 = nc.dram_tensor("b1c", [64, 1], F32, kind="ExternalInput")
    d_b2 = nc.dram_tensor("b2c", [32, 1], F32, kind="ExternalInput")
    d_b3 = nc.dram_tensor("b3c", [1, 1], F32, kind="ExternalInput")

    d_y = nc.dram_tensor("y", [S, 1], F32, kind="ExternalOutput")

    # d_qblB carries one extra fence row (bw): written only after group 0's
    # q gathers are issued, so the B exchange cannot jump ahead of them on
    # the Pool engine's ready queue and delay group 0's scores.
    d_qblA = nc.dram_tensor("qb_localA", [aw, N], F32)
    d_qblB = nc.dram_tensor("qb_localB", [bw + 1, N], F32)
    d_qbaA = nc.dram_tensor("qb_allA", [ND * aw, N], F32, addr_space="Shared")
    d_qbaB = nc.dram_tensor("qb_allB", [ND * (bw + 1), N], F32,
                            addr_space="Shared")
    d_clg = [nc.dram_tensor(f"cand_local{g}", [GPOS, CW16], U32)
             for g in range(NG)]
    d_cag = [nc.dram_tensor(f"cand_all{g}", [ND * GPOS, CW16], U32,
                            addr_space="Shared") for g in range(NG)]

    groups = [list(range(ND))]

    with tile.TileContext(nc) as tc:
        cpool = tc.alloc_tile_pool(name="const", bufs=1)
        big = tc.alloc_tile_pool(name="big", bufs=1)

        # ---- constants / params to SBUF ----
        ident = cpool.tile([128, 128], F32)
        make_identity(nc, ident[:])
        ones1 = cpool.tile([1, 128], F32)
        nc.vector.memset(ones1[:], 1.0)
        onesf = cpool.tile([128, 128], F32)
        nc.vector.memset(onesf[:], 1.0)

        def load(pool, dram, shape, dtype=F32):
            t = pool.tile(shape, dtype, tag=f"ld_{dram.name}")
            nc.sync.dma_start(out=t[:], in_=dram[:, :])
            return t

        # LSTM weights and the ppre inputs first (the SP DMA queue is
        # in-order; PE's first scheduled ops are the ppre matmuls and the
        # first LSTM step, which need these plus only the first xt chunk)
        wih = load(cpool, d_wih, [F + 1, 4 * N])
        whh = load(cpool, d_whh, [N, 4 * N])
        nuq = load(cpool, d_nuq, [1, N])
        nuk = load(cpool, d_nuk, [1, N])
        # time-major xt arrives in XCH separate chunk tiles so the LSTM's
        # step-t matmul depends only on its own chunk's DMA
        TPC = L // XCH
        CW = TPC * SS
        xts = []
        for c in range(XCH):
            xtc = big.tile([F + 1, CW], F32, tag=f"xt{c}")
            nc.sync.dma_start(out=xtc[:], in_=d_xt[:, c * CW:(c + 1) * CW])
            xts.append(xtc)
        wqt = load(cpool, d_wqt, [N, N])
        wkt = load(cpool, d_wkt, [N, N])
        invt = load(cpool, d_invt, [1, 1])
        trowsA = load(cpool, d_trowsA, [128, NT], U32)
        trowsB = load(cpool, d_trowsB, [128, NT], U32)
        mrows = load(cpool, d_mrows, [128, NG * ND], U32)
        orows = load(cpool, d_orows, [128, NG], U32)
        w1t = load(cpool, d_w1t, [2 * F, 64])
        w2t = load(cpool, d_w2t, [64, 32])
        w3t = load(cpool, d_w3t, [32, 1])
        b1c = load(cpool, d_b1, [64, 1])
        b2c = load(cpool, d_b2, [32, 1])
        b3c = load(cpool, d_b3, [1, 1])

        gslo = load(cpool, d_gslo, [128, NG])
        invtb = cpool.tile([128, 1], F32)
        nc.gpsimd.partition_broadcast(invtb[:], invt[:], channels=128)

        iota_u = cpool.tile([128, 64], U32)
        nc.gpsimd.iota(iota_u[:], pattern=[[1, 64]], base=0, channel_multiplier=0)
        iota_f = cpool.tile([128, 64], F32)
        nc.vector.tensor_copy(iota_f[:], iota_u[:])
        base_u = cpool.tile([128, 64], U32)
        nc.gpsimd.iota(base_u[:], pattern=[[COLS, 8], [0, 8]], base=0,
                       channel_multiplier=0)
        base_f = cpool.tile([128, 64], F32)
        nc.vector.tensor_copy(base_f[:], base_u[:])

        # rank-1 LN-fold correction matrices: rows n, cols p -> -u[p]/128
        with tc.tile_pool(name="ppre", bufs=1, space="PSUM") as ppre:
            uqo = cpool.tile([128, 128], F32)
            uko = cpool.tile([128, 128], F32)
            pq = ppre.tile([128, 128], F32, space="PSUM")
            nc.tensor.matmul(out=pq[:], lhsT=ones1[:], rhs=nuq[:], start=True, stop=True)
            nc.scalar.activation(uqo[:], pq[:], AF.Copy)
            pk = ppre.tile([128, 128], F32, space="PSUM")
            nc.tensor.matmul(out=pk[:], lhsT=ones1[:], rhs=nuk[:], start=True, stop=True)
            nc.scalar.activation(uko[:], pk[:], AF.Copy)

        # ---- Phase 1: LSTM over the 375 local stocks ----
        # h,c layout [n=128, s]; last-10 hidden states land in hsave[n, s*10+k].
        # Gate columns in wih/whh are host-permuted to [i, f, o, g]; the bias
        # is folded into the xproj matmul via xt's constant-1 row.
        hsave = big.tile([128, COLS], F32)
        with tc.tile_pool(name="lstm_sb", bufs=2) as lsb, \
             tc.tile_pool(name="lstm_c", bufs=2) as lcp, \
             tc.tile_pool(name="lstm_ps", bufs=2, space="PSUM") as lps:
            h0 = lsb.tile([128, SS], F32, tag="h0")
            nc.vector.memset(h0[:], 0.0)
            c_prev = lcp.tile([128, SS], F32, tag="c")
            nc.vector.memset(c_prev[:], 0.0)
            h_prev = h0[:]
            for t in range(L):
                xs = xts[t // TPC][:, (t % TPC) * SS:(t % TPC + 1) * SS]
                # all four x-projections first: they don't depend on h_{t-1},
                # so the PE has work while the gate elementwise chain finishes
                gps = []
                for g in range(4):
                    psg = lps.tile([128, SS], F32, tag=f"g{g}", space="PSUM")
                    nc.tensor.matmul(out=psg[:], lhsT=wih[:, g * N:(g + 1) * N],
                                     rhs=xs, start=True, stop=False)
                    gps.append(psg)
                for g in range(4):
                    nc.tensor.matmul(out=gps[g][:], lhsT=whh[:, g * N:(g + 1) * N],
                                     rhs=h_prev, start=False, stop=True)
                si = lsb.tile([128, SS], F32, tag="si")
                nc.scalar.activation(si[:], gps[0][:], AF.Sigmoid)
                sf = lsb.tile([128, SS], F32, tag="sf")
                nc.scalar.activation(sf[:], gps[1][:], AF.Sigmoid)
                so_t = lsb.tile([128, SS], F32, tag="so")
                nc.scalar.activation(so_t[:], gps[2][:], AF.Sigmoid)
                tg = lsb.tile([128, SS], F32, tag="tg")
                nc.scalar.activation(tg[:], gps[3][:], AF.Tanh)
                si, sf, so = si[:], sf[:], so_t[:]
                t1 = lsb.tile([128, SS], F32, tag="t1")
                nc.vector.tensor_tensor(out=t1[:], in0=si, in1=tg[:], op=ALU.mult)
                c2 = lsb.tile([128, SS], F32, tag="c2")
                nc.gpsimd.tensor_tensor(out=c2[:], in0=sf, in1=c_prev[:], op=ALU.mult)
                c_new = lcp.tile([128, SS], F32, tag="c")
                nc.vector.tensor_tensor(out=c_new[:], in0=c2[:], in1=t1[:], op=ALU.add)
                th = lsb.tile([128, SS], F32, tag="th")
                nc.scalar.activation(th[:], c_new[:], AF.Tanh)
                if t >= L - LMAX:
                    h_out = hsave[:, (t - (L - LMAX))::LMAX]
                    nc.gpsimd.tensor_tensor(out=h_out, in0=so, in1=th[:], op=ALU.mult)
                    h_prev = h_out
                else:
                    hn = lsb.tile([128, SS], F32, tag="hn")
                    nc.gpsimd.tensor_tensor(out=hn[:], in0=so, in1=th[:], op=ALU.mult)
                    h_prev = hn[:]
                c_prev = c_new

        # ---- Phase 2: queries first, so the AllGather overlaps the keys ----
        with tc.tile_pool(name="q_sb", bufs=2) as qsb, \
             tc.tile_pool(name="q_ps", bufs=2, space="PSUM") as qps:
            h39 = hsave[:, (LMAX - 1)::LMAX]
            pyq = qps.tile([128, SS], F32, tag="yq", space="PSUM")
            nc.tensor.matmul(out=pyq[:], lhsT=wqt[:], rhs=h39, start=True, stop=False)
            nc.tensor.matmul(out=pyq[:], lhsT=uqo[:], rhs=h39, start=False, stop=True)
            yq = qsb.tile([128, SS], F32, tag="yq_sb")
            nc.scalar.activation(yq[:], pyq[:], AF.Copy)
            y2q = qsb.tile([128, SS], F32, tag="y2q")
            nc.scalar.activation(y2q[:], pyq[:], AF.Square)
            psq = qps.tile([128, SS], F32, tag="sq", space="PSUM")
            nc.tensor.matmul(out=psq[:], lhsT=onesf[:], rhs=y2q[:], start=True, stop=True)
            sq = qsb.tile([128, SS], F32, tag="sqq")
            nc.scalar.sqrt(sq[:], psq[:])
            ri = qsb.tile([128, SS], F32, tag="riq")
            nc.vector.reciprocal(ri[:], sq[:])
            qn1 = qsb.tile([128, SS], F32, tag="qn1")
            nc.vector.tensor_tensor(out=qn1[:], in0=yq[:], in1=ri[:], op=ALU.mult)
            qn = qsb.tile([128, SS], F32, tag="qn")
            nc.vector.tensor_scalar(out=qn[:], in0=qn1[:], scalar1=invtb[:, 0:1],
                                    scalar2=None, op0=ALU.mult)
            for j in range(3):
                pt = qps.tile([128, 128], F32, tag="qt", space="PSUM")
                nc.tensor.transpose(out=pt[0:MTS, :], in_=qn[:, j * MTS:(j + 1) * MTS],
                                    identity=ident[:])
                qrow_sb = qsb.tile([128, 128], F32, tag="qrow_sb")
                nc.scalar.activation(qrow_sb[0:MTS, :], pt[0:MTS, :], AF.Copy)
                # split the q rows at local stock index `aw` between the
                # early (A) and late (B) exchange tensors
                lo, hi = j * MTS, j * MTS + MTS
                if hi <= aw:
                    nc.sync.dma_start(out=d_qblA[lo:hi, :], in_=qrow_sb[0:MTS, :])
                elif lo >= aw:
                    nc.sync.dma_start(out=d_qblB[lo - aw:hi - aw, :],
                                      in_=qrow_sb[0:MTS, :])
                else:
                    na = aw - lo
                    nc.sync.dma_start(out=d_qblA[lo:aw, :], in_=qrow_sb[0:na, :])
                    nc.sync.dma_start(out=d_qblB[0:hi - aw, :],
                                      in_=qrow_sb[na:MTS, :])
                if hi >= aw and lo < aw:
                    # all A rows written: fire the early exchange now
                    nc.gpsimd.collective_compute(
                        "AllGather", ALU.bypass, replica_groups=groups,
                        ins=[d_qblA.ap().opt()], outs=[d_qbaA.ap().opt()])

        # ---- Phase 3: keys (LN+l2norm folded into matmuls; sigma cancels) ----
        keysT = big.tile([128, COLS], F32)
        with tc.tile_pool(name="key_sb", bufs=3) as ksb, \
             tc.tile_pool(name="key_ps", bufs=2, space="PSUM") as kps:
            ysb = big.tile([128, COLS], F32)
            CH = 512
            chunks = [(c0, min(COLS, c0 + CH)) for c0 in range(0, COLS, CH)]
            sqs = []
            for c0, c1 in chunks:
                w = c1 - c0
                py = kps.tile([128, CH], F32, tag="y", space="PSUM")
                nc.tensor.matmul(out=py[:, :w], lhsT=wkt[:],
                                 rhs=hsave[:, c0:c1], start=True, stop=False)
                nc.tensor.matmul(out=py[:, :w], lhsT=uko[:],
                                 rhs=hsave[:, c0:c1], start=False, stop=True)
                nc.scalar.activation(ysb[:, c0:c1], py[:, :w], AF.Copy)
                y2 = ksb.tile([128, CH], F32, tag="y2")
                nc.scalar.activation(y2[:, :w], py[:, :w], AF.Square)
                psq = kps.tile([128, CH], F32, tag="s", space="PSUM")
                nc.tensor.matmul(out=psq[:, :w], lhsT=onesf[:],
                                 rhs=y2[:, :w], start=True, stop=True)
                sq = ksb.tile([128, CH], F32, tag="sq")
                nc.scalar.sqrt(sq[:, :w], psq[:, :w])
                sqs.append((sq, c0, c1))
            for sq, c0, c1 in sqs:
                w = c1 - c0
                ri = ksb.tile([128, CH], F32, tag="ri")
                nc.vector.reciprocal(ri[:, :w], sq[:, :w])
                nc.vector.tensor_tensor(out=keysT[:, c0:c1], in0=ysb[:, c0:c1],
                                        in1=ri[:, :w], op=ALU.mult)

        # ---- Phase 4+5+6: per group: qT gather/transpose + scores + top-8
        # (per tile), candidate AllGather exchange, and the merge of the
        # PREVIOUS group's exchanged candidates ----
        qT_all = big.tile([128, NPOS], F32)
        v8 = big.tile([128, NT * 8], F32)
        i8 = big.tile([128, NT * 8], U32)
        cl_v = [d_clg[g].ap().rearrange("(T p) w -> p T w", p=128)
                for g in range(NG)]

        qrow0 = big.tile([128, GT * 128], F32)
        gsb = tc.alloc_tile_pool(name="qg_sb", bufs=8)
        gps_ = tc.alloc_tile_pool(name="qg_ps", bufs=2, space="PSUM")
        ssb_p = tc.alloc_tile_pool(name="sc_sb", bufs=2)
        sps = tc.alloc_tile_pool(name="sc_ps", bufs=2, space="PSUM")
        msb = tc.alloc_tile_pool(name="m_sb", bufs=2)
        mps = tc.alloc_tile_pool(name="m_ps", bufs=1, space="PSUM")

        def merge_group(g):
            # candidates for my 125 targets of group g, from all 8 cores
            mv = msb.tile([128, ND * CW16], U32, tag="mv")
            for s in range(ND):
                nc.gpsimd.indirect_dma_start(
                    out=mv[0:MTS, s * CW16:(s + 1) * CW16], out_offset=None,
                    in_=d_cag[g][:, :],
                    in_offset=IndirectOffsetOnAxis(
                        ap=mrows[0:MTS, g * ND + s:g * ND + s + 1], axis=0))
            mvals = mv[0:MTS, :].bitcast(F32).rearrange(
                "p (d w) -> p d w", w=CW16)[:, :, 0:8]
            midx = mv[0:MTS, :].rearrange("p (d w) -> p d w", w=CW16)[:, :, 8:16]
            # leading copies + self-filter arithmetic on gpsimd so the DVE's
            # scan stream is disturbed as little as possible
            mvalc = msb.tile([128, 64], F32, tag="mvalc")
            nc.gpsimd.tensor_copy(mvalc[0:MTS, :], mvals)
            # global flat candidate index = owner*COLS + local
            midxf = msb.tile([128, 64], F32, tag="midxf")
            nc.gpsimd.tensor_copy(midxf[0:MTS, :], midx)
            nc.gpsimd.tensor_tensor(out=midxf[0:MTS, :], in0=midxf[0:MTS, :],
                                    in1=base_f[0:MTS, :], op=ALU.add)
            # self-stock filter (candidates were exchanged unmasked):
            # self iff |gidx - (10*self_stock + 4.5)| < 5
            dm = msb.tile([128, 64], F32, tag="dm")
            nc.gpsimd.tensor_tensor(
                out=dm[0:MTS, :], in0=midxf[0:MTS, :],
                in1=gslo[0:MTS, g:g + 1].to_broadcast([MTS, 64]),
                op=ALU.subtract)
            nc.gpsimd.tensor_tensor(out=dm[0:MTS, :], in0=dm[0:MTS, :],
                                    in1=dm[0:MTS, :], op=ALU.mult)
            nc.vector.tensor_scalar(out=dm[0:MTS, :], in0=dm[0:MTS, :],
                                    scalar1=25.0, scalar2=None, op0=ALU.is_lt)
            nc.vector.scalar_tensor_tensor(out=mvalc[0:MTS, :], in0=dm[0:MTS, :],
                                           scalar=-1e30, in1=mvalc[0:MTS, :],
                                           op0=ALU.mult, op1=ALU.add)
            v8m = msb.tile([128, 8], F32, tag="v8m")
            nc.vector.max(out=v8m[0:MTS, :], in_=mvalc[0:MTS, :])
            pos8 = msb.tile([128, 8], U32, tag="pos8")
            nc.vector.max_index(out=pos8[0:MTS, :], in_max=v8m[0:MTS, :],
                                in_values=mvalc[0:MTS, :])
            pos5f = msb.tile([128, 5], F32, tag="pos5f")
            nc.gpsimd.tensor_copy(pos5f[0:MTS, :], pos8[0:MTS, 0:5])
            eq = msb.tile([128, 5 * 64], F32, tag="eq")
            eq_v = eq[0:MTS, :].rearrange("p (k c) -> p k c", c=64)
            nc.vector.tensor_tensor(
                out=eq_v,
                in0=pos5f[0:MTS, :].rearrange("p k -> p k ()").to_broadcast([MTS, 5, 64]),
                in1=iota_f[0:MTS, :].rearrange("p c -> p () c").to_broadcast([MTS, 5, 64]),
                op=ALU.is_equal)
            nc.vector.tensor_tensor(
                out=eq_v, in0=eq_v,
                in1=midxf[0:MTS, :].rearrange("p c -> p () c").to_broadcast([MTS, 5, 64]),
                op=ALU.mult)
            g5f = msb.tile([128, 5], F32, tag="g5f")
            nc.vector.tensor_reduce(out=g5f[0:MTS, :], in_=eq_v,
                                    axis=mybir.AxisListType.X, op=ALU.add)
            g5u = msb.tile([128, 5], U32, tag="g5u")
            nc.vector.tensor_copy(g5u[0:MTS, :], g5f[0:MTS, :])
            # z rows (6 raw feats + lag_bias + pad) for the 5 winners
            zb = msb.tile([128, 5 * 8], F32, tag="zb")
            for r in range(K):
                nc.gpsimd.indirect_dma_start(
                    out=zb[0:MTS, r * 8:(r + 1) * 8], out_offset=None,
                    in_=d_xzb[:, :],
                    in_offset=IndirectOffsetOnAxis(ap=g5u[0:MTS, r:r + 1], axis=0))
            vb5 = msb.tile([128, 5], F32, tag="vb5")
            nc.vector.tensor_tensor(out=vb5[0:MTS, :], in0=v8m[0:MTS, 0:5],
                                    in1=zb[0:MTS, 6::8], op=ALU.add)
            # softmax over the 5 candidate scores
            mx = msb.tile([128, 1], F32, tag="mx")
            nc.vector.tensor_reduce(out=mx[0:MTS, :], in_=vb5[0:MTS, :],
                                    axis=mybir.AxisListType.X, op=ALU.max)
            nmx = msb.tile([128, 1], F32, tag="nmx")
            nc.vector.tensor_scalar(out=nmx[0:MTS, :], in0=mx[0:MTS, :],
                                    scalar1=-1.0, scalar2=None, op0=ALU.mult)
            e5 = msb.tile([128, 5], F32, tag="e5")
            nc.scalar.activation(e5[0:MTS, :], vb5[0:MTS, :], AF.Exp,
                                 bias=nmx[0:MTS, 0:1])
            ssum = msb.tile([128, 1], F32, tag="ssum")
            nc.vector.tensor_reduce(out=ssum[0:MTS, :], in_=e5[0:MTS, :],
                                    axis=mybir.AxisListType.X, op=ALU.add)
            rs = msb.tile([128, 1], F32, tag="rs")
            nc.vector.reciprocal(rs[0:MTS, :], ssum[0:MTS, :])
            w5 = msb.tile([128, 5], F32, tag="w5")
            nc.vector.tensor_scalar(out=w5[0:MTS, :], in0=e5[0:MTS, :],
                                    scalar1=rs[0:MTS, 0:1], scalar2=None, op0=ALU.mult)
            # z_agg = sum_r w_r * z_r ; feat = [z_agg, z_0]
            wz = msb.tile([128, 5 * 6], F32, tag="wz")
            zview = zb[0:MTS, :].rearrange("p (r w) -> p r w", w=8)[:, :, 0:6]
            nc.vector.tensor_tensor(
                out=wz[0:MTS, :].rearrange("p (r f) -> p r f", f=6),
                in0=zview,
                in1=w5[0:MTS, :].rearrange("p r -> p r ()").to_broadcast([MTS, 5, 6]),
                op=ALU.mult)
            feat = msb.tile([128, 2 * F], F32, tag="feat")
            nc.vector.tensor_reduce(
                out=feat[0:MTS, 0:6],
                in_=wz[0:MTS, :].rearrange("p (r f) -> p f r", f=6),
                axis=mybir.AxisListType.X, op=ALU.add)
            nc.gpsimd.tensor_copy(feat[0:MTS, 6:12], zb[0:MTS, 0:6])
            # MLP head (tiny fp32 matmuls; PSUM tags reused down the chain)
            pft = mps.tile([128, 128], F32, tag="mlpA", space="PSUM")
            nc.tensor.transpose(out=pft[0:2 * F, 0:128], in_=feat[:, :],
                                identity=ident[:])
            featT = msb.tile([2 * F, 128], F32, tag="featT")
            nc.scalar.activation(featT[:], pft[0:2 * F, 0:128], AF.Copy)
            ph1 = mps.tile([128, 128], F32, tag="mlpB", space="PSUM")
            nc.tensor.matmul(out=ph1[0:64, :], lhsT=w1t[:], rhs=featT[:],
                             start=True, stop=True)
            h1 = msb.tile([64, 128], F32, tag="h1")
            nc.scalar.activation(h1[:], ph1[0:64, :], AF.Relu, bias=b1c[:, 0:1])
            ph2 = mps.tile([128, 128], F32, tag="mlpA", space="PSUM")
            nc.tensor.matmul(out=ph2[0:32, :], lhsT=w2t[:], rhs=h1[:],
                             start=True, stop=True)
            h2 = msb.tile([32, 128], F32, tag="h2")
            nc.scalar.activation(h2[:], ph2[0:32, :], AF.Relu, bias=b2c[:, 0:1])
            py_ = mps.tile([128, 128], F32, tag="mlpB", space="PSUM")
            nc.tensor.matmul(out=py_[0:1, :], lhsT=w3t[:], rhs=h2[:],
                             start=True, stop=True)
            yrow = msb.tile([1, 128], F32, tag="yrow")
            nc.scalar.activation(yrow[:], py_[0:1, :], AF.Identity, bias=b3c[0:1, 0:1])
            pyt = mps.tile([128, 128], F32, tag="mlpA", space="PSUM")
            nc.tensor.transpose(out=pyt[:, 0:1], in_=yrow[0:1, :],
                                identity=ident[0:1, 0:1])
            ycol = msb.tile([128, 1], F32, tag="ycol")
            nc.vector.tensor_copy(ycol[:], pyt[:, 0:1])
            nc.gpsimd.indirect_dma_start(
                out=d_y[:, :],
                out_offset=IndirectOffsetOnAxis(ap=orows[0:MTS, g:g + 1], axis=0),
                in_=ycol[0:MTS, :], in_offset=None)

        for g in range(NG):
            for T8 in range(GT):
                T = GT * g + T8
                # per-tile transposed queries (gathers pipeline on Pool).
                # Group 0's targets reference only A-rows by construction;
                # later tiles mix A and B rows, filled by two bounds-checked
                # gathers (out-of-bounds offsets are skipped silently).
                if g == 0:
                    # group 0 gathers land in one dedicated wide tile: no
                    # buffer recycling against the PE transposes, so all 8
                    # gathers and the fence run back-to-back right after the
                    # early q exchange
                    qrow = qrow0[:, T8 * 128:(T8 + 1) * 128]
                    nc.gpsimd.indirect_dma_start(
                        out=qrow, out_offset=None, in_=d_qbaA[:, :],
                        in_offset=IndirectOffsetOnAxis(ap=trowsA[:, T:T + 1], axis=0))
                else:
                    qrowt = gsb.tile([128, 128], F32, tag="qrow")
                    qrow = qrowt[:]
                    nc.gpsimd.indirect_dma_start(
                        out=qrow, out_offset=None, in_=d_qbaA[:, :],
                        in_offset=IndirectOffsetOnAxis(ap=trowsA[:, T:T + 1], axis=0),
                        bounds_check=ND * aw - 1, oob_is_err=False)
                    nc.gpsimd.indirect_dma_start(
                        out=qrow, out_offset=None, in_=d_qbaB[:, :],
                        in_offset=IndirectOffsetOnAxis(ap=trowsB[:, T:T + 1], axis=0),
                        bounds_check=ND * (SS - aw + 1) - 1, oob_is_err=False)
                if g == 0 and T8 == GT - 1:
                    # all of group 0's gathers are issued: write the fence row
                    # (data-dependent on the last gather), then fire the late
                    # q exchange (completes well before group 1 needs it)
                    fence = gsb.tile([1, N], F32, tag="fence")
                    nc.gpsimd.tensor_copy(
                        fence[:], qrow0[0:1, (GT - 1) * 128:GT * 128])
                    nc.sync.dma_start(out=d_qblB[bw:bw + 1, :], in_=fence[:])
                    nc.gpsimd.collective_compute(
                        "AllGather", ALU.bypass, replica_groups=groups,
                        ins=[d_qblB.ap().opt()], outs=[d_qbaB.ap().opt()])
                pt = gps_.tile([128, 128], F32, tag="pt", space="PSUM")
                nc.tensor.transpose(out=pt[:], in_=qrow, identity=ident[:])
                qT = qT_all[:, T * 128:(T + 1) * 128]
                nc.scalar.activation(qT, pt[:], AF.Copy)
                ssb = ssb_p.tile([128, COLS], F32, tag="ssb")
                for c0 in range(0, COLS, SCH):
                    c1 = min(COLS, c0 + SCH)
                    ps = sps.tile([128, SCH], F32, tag="sc", space="PSUM")
                    for b0 in range(0, c1 - c0, 512):
                        b1 = min(c1 - c0, b0 + 512)
                        nc.tensor.matmul(out=ps[:, b0:b1], lhsT=qT,
                                         rhs=keysT[:, c0 + b0:c0 + b1],
                                         start=True, stop=True)
                    nc.scalar.activation(ssb[:, c0:c1], ps[:, 0:c1 - c0], AF.Copy)
                nc.vector.max(out=v8[:, T * 8:(T + 1) * 8], in_=ssb[:])
                nc.vector.max_index(out=i8[:, T * 8:(T + 1) * 8],
                                    in_max=v8[:, T * 8:(T + 1) * 8], in_values=ssb[:])
            # merge previous group's exchanged candidates (its AllGather has
            # had a full group of score tiles to complete in)
            if g >= 1:
                merge_group(g - 1)
            # pack + exchange this group's candidates UNMASKED (the self
            # filter runs on the merge side, so the exchange depends only on
            # the scans and fires immediately)
            gsl = slice(g * GT * 8, (g + 1) * GT * 8)
            nc.sync.dma_start(
                out=cl_v[g][:, :, 0:8],
                in_=v8[:, gsl].rearrange("p (T w) -> p T w", w=8).bitcast(U32))
            nc.sync.dma_start(
                out=cl_v[g][:, :, 8:16],
                in_=i8[:, gsl].rearrange("p (T w) -> p T w", w=8))
            nc.gpsimd.collective_compute(
                "AllGather", ALU.bypass, replica_groups=groups,
                ins=[d_clg[g].ap().opt()], outs=[d_cag[g].ap().opt()])
        merge_group(NG - 1)

        mps.release()
        msb.release()
        sps.release()
        ssb_p.release()
        gps_.release()
        gsb.release()
        big.release()
        cpool.release()

    nc.compile()
    return nc


_CACHED_NC = {}
_BIGROW = 10_000_000


def _assignment(tix):
    """Per-owner target->(group, slot) assignment: each owner's 375 targets
    sorted by the referenced stock's local index lam = tix % SS; group g gets
    the g-th 125 of that order (so group 0 needs only low-lam q rows), and
    the A/B split threshold aw covers every group-0 lam."""
    asg = np.zeros((ND, NG, MTS), np.int64)
    aw = 126
    for d in range(ND):
        ts = np.arange(d * SS, (d + 1) * SS)
        order = np.argsort(tix[ts] % SS, kind="stable")
        for gg in range(NG):
            asg[d, gg] = ts[order[gg * MTS:(gg + 1) * MTS]]
        aw = max(aw, int((tix[asg[d, 0]] % SS).max()) + 1)
    assert aw <= 250, f"group-0 lam spread too wide: {aw}"
    return asg, aw


def _prep_inputs(X_scaled, X_raw, target_idx, lstm_Wih, lstm_Whh, lstm_bih,
                 lstm_bhh, ln_g, ln_b, WQ, WK, log_temp, lag_bias,
                 W1, b1, W2, b2, W3, b3):
    f32 = np.float32
    assert np.all(np.asarray(ln_b) == 0.0), "kernel assumes ln_b == 0"
    tix = np.asarray(target_idx).astype(np.int64)
    bias = (np.asarray(lstm_bih) + np.asarray(lstm_bhh)).astype(f32)
    gperm = np.r_[0:N, N:2 * N, 3 * N:4 * N, 2 * N:3 * N]    # [i, f, o, g]
    g_ln = np.asarray(ln_g).astype(f32)
    wq_f = (np.asarray(WQ) * g_ln[None, :]).astype(f32)
    wk_f = (np.asarray(WK) * g_ln[None, :]).astype(f32)
    uq = np.asarray(WQ) @ g_ln
    uk = np.asarray(WK) @ g_ln
    inv_temp = np.asarray(
        1.0 / np.clip(np.exp(np.asarray(log_temp, np.float64)), 0.1, np.sqrt(N)),
        f32).reshape(1, 1)

    # XZB table: flat (s,l) -> [6 raw feats at lag_pos, lag_bias, 0]
    Xr = np.asarray(X_raw)[0].astype(f32)                    # [S, L, F]
    lb = np.asarray(lag_bias).astype(f32)
    xzb = np.zeros((S * LMAX, 8), f32)
    lagpos = np.clip(L - 1 - (LMAX - np.arange(LMAX)), 0, L - 1)
    xzb[:, 0:6] = Xr[:, lagpos, :].reshape(S * LMAX, 6)
    xzb[:, 6] = np.tile(lb, S)

    # lam-sorted position assignment; target asg[d, g, o] sits at position
    # g*1024 + d*128 + o (tile T = 8g + d, row o)
    asg, aw = _assignment(tix)
    bw = SS - aw
    trowsA = np.full((128, NT), _BIGROW, np.uint32)
    trowsB = np.full((128, NT), _BIGROW, np.uint32)
    trowsA[:, 0:GT] = 0  # group-0 pad rows still gather a valid row
    gslo = np.full((128, NG, ND), -1e9, f32)
    orows = np.zeros((128, NG, ND), np.uint32)
    mrows = np.zeros((128, NG * ND, ND), np.uint32)
    for d in range(ND):
        for gg in range(NG):
            for o in range(MTS):
                t = asg[d, gg, o]
                sig = int(tix[t])
                ow, lam = divmod(sig, SS)
                T = GT * gg + d
                if lam < aw:
                    trowsA[o, T] = ow * aw + lam
                    trowsB[o, T] = _BIGROW
                else:
                    assert gg > 0
                    trowsA[o, T] = _BIGROW
                    trowsB[o, T] = ow * (bw + 1) + (lam - aw)
                gslo[o, gg, d] = sig * LMAX + 4.5
                orows[o, gg, d] = t
            for s in range(ND):
                mrows[:MTS, gg * ND + s, d] = s * GPOS + d * 128 + np.arange(MTS)

    Xs = np.asarray(X_scaled)[0].astype(f32)                 # [S, L, F]
    common = dict(
        wih_t=np.ascontiguousarray(np.vstack([
            np.asarray(lstm_Wih).astype(f32).T, bias[None, :]])[:, gperm]),
        whh_t=np.ascontiguousarray(np.asarray(lstm_Whh).astype(f32).T[:, gperm]),
        wq_t=np.ascontiguousarray(wq_f.T), wk_t=np.ascontiguousarray(wk_f.T),
        negu_q=np.ascontiguousarray((-uq.astype(f32) / N).reshape(1, N)),
        negu_k=np.ascontiguousarray((-uk.astype(f32) / N).reshape(1, N)),
        invt=inv_temp, xzb=xzb, tgtrowsA=trowsA, tgtrowsB=trowsB,
        w1_t=np.ascontiguousarray(np.asarray(W1).astype(f32).T),
        w2_t=np.ascontiguousarray(np.asarray(W2).astype(f32).T),
        w3_t=np.ascontiguousarray(np.asarray(W3).astype(f32).T),
        b1c=np.asarray(b1).astype(f32).reshape(64, 1),
        b2c=np.asarray(b2).astype(f32).reshape(32, 1),
        b3c=np.asarray(b3).astype(f32).reshape(1, 1),
    )
    in_maps = []
    for d in range(ND):
        # time-major xt: [F+1, L*SS], column = t*SS + s
        xtv = np.ascontiguousarray(np.vstack([
            Xs[d * SS:(d + 1) * SS].transpose(2, 1, 0).reshape(F, L * SS),
            np.ones((1, L * SS), f32)]))
        in_maps.append(dict(
            common, xt=xtv,
            gslo=np.ascontiguousarray(gslo[:, :, d]),
            mrows=np.ascontiguousarray(mrows[:, :, d]),
            orows=np.ascontiguousarray(orows[:, :, d]),
        ))
    return in_maps


def kernel(**inputs):
    tix = np.asarray(inputs["target_idx"]).astype(np.int64)
    _, aw = _assignment(tix)
    if aw not in _CACHED_NC:
        _CACHED_NC[aw] = build_program(aw)
    nc = _CACHED_NC[aw]
    in_maps = _prep_inputs(**inputs)
    res = run_bass_kernel_spmd(nc, in_maps, core_ids=list(range(ND)))
    y = np.zeros(S, np.float32)
    for d in range(ND):
        y[d * SS:(d + 1) * SS] = res.results[d]["y"][d * SS:(d + 1) * SS, 0]
    return y
